# revision 7
# baseline (speedup 1.0000x reference)
"""Trainium2 Bass kernel for nn_ActorCritic forward (8-core tensor-parallel).

Strategy (memory-bound problem — ~573MB of head weights dominate):
  - The trunk (card embedding + shared transformer block over the 55
    concatenated hand/draw/disc tokens with block-diagonal attention masking,
    other-MLP, global feature assembly + layernorms) is small (~11MB of
    weights) and is computed replicated on all 8 cores.
  - The four head MLPs (ac: 4608^2 x2, am/ae/cr: 4096^2 x2) are sharded
    column-parallel across the 8 cores: each core computes a 1/8 slice of
    h1 = relu(W1 x + b1), an AllGather rebuilds the full h1 on every core,
    each core computes its slice of h2 = relu(W2 h1 + b2), a second
    AllGather rebuilds h2, and the tiny W3 projections + masked softmax
    run redundantly on every core (host reads core 0).
  - All big matmuls put the weights on the MOVING operand (rhs) of the PE
    so weight bytes stream HBM->SBUF->PE at full DMA rate; the stationary
    operand is the tiny activation vector/matrix (<=10 columns).
  - The lnac affine (gamma/beta over the 4608-dim ac head input) is folded
    into the ac W1 weights/bias on the host, so the device only has to
    normalize (saves two [10,4608] broadcast tiles of SBUF).

kernel(**inputs) takes the FULL unsharded inputs (as reference.setup_inputs
produces) and returns (probs[22], value[1]) like reference.reference.
"""

import numpy as np
from contextlib import ExitStack

import concourse.bass as bass
import concourse.bacc as bacc
import concourse.tile as tile
from concourse import mybir
from concourse.bass_utils import run_bass_kernel_spmd
from concourse.masks import make_identity

F32 = mybir.dt.float32
AF = mybir.ActivationFunctionType
ALU = mybir.AluOpType
AXX = mybir.AxisListType

NCORES = 8
D = 512
S_HAND, S_DRAW, S_DISC = 10, 30, 15
S = S_HAND + S_DRAW + S_DISC          # 55
NH, HD = 8, 64
EPS = 1e-5
D_AC = 9 * D                           # 4608
D_G = 8 * D                            # 4096
SH_AC = D_AC // NCORES                 # 576
SH_G = D_G // NCORES                   # 512
NEG = -1e30
BN = 7296                              # bounce size: 10*576 + 3*512
OFF_AC, OFF_AM, OFF_AE, OFF_CR = 0, 5760, 6272, 6784
# ac K-tiling of a gathered [10,576] block: 128,128,128,128,64
AC_BLK = [(j * 128, min(128, SH_AC - j * 128)) for j in range((SH_AC + 127) // 128)]
# global-dim (row, size) K-tiles, in gather order, for the ac W2/W3 stages
AC_TILES = [(c * SH_AC + off, sz) for c in range(NCORES) for (off, sz) in AC_BLK]

INPUT_SPEC = {
    'cardsT': (128, S), 'sel': (S, 6), 'maskfull': (S, S), 'maskbias22': (1, 22),
    'otherT': (120, 1),
    'emb_wT': (128, D), 'emb_b': (1, D),
    'inproj_wT': (D, 3 * D), 'inproj_b': (1, 3 * D),
    'outproj_wT': (D, D), 'outproj_b': (1, D),
    'ln1_g': (D,), 'ln1_b': (D,),
    'ff1_wT': (D, D), 'ff1_b': (1, D),
    'ff2_wT': (D, D), 'ff2_b': (1, D),
    'ln2_g': (D,), 'ln2_b': (D,),
    'other_w1T': (120, 2 * D), 'other_b1': (1, 2 * D),
    'other_w2T': (2 * D, 2 * D), 'other_b2': (1, 2 * D),
    'lng_g': (D_G,), 'lng_b': (D_G,),
    'ac_w1T_s': (D_AC, SH_AC), 'ac_b1_s': (1, SH_AC),
    'ac_w2T_s': (D_AC, SH_AC), 'ac_b2_s': (1, SH_AC),
    'ac_w3T': (D_AC, 2), 'ac_b3': (1, 2),
    'am_w1T_s': (D_G, SH_G), 'am_b1_s': (1, SH_G),
    'am_w2T_s': (D_G, SH_G), 'am_b2_s': (1, SH_G),
    'am_w3T': (D_G, 1), 'am_b3': (1, 1),
    'ae_w1T_s': (D_G, SH_G), 'ae_b1_s': (1, SH_G),
    'ae_w2T_s': (D_G, SH_G), 'ae_b2_s': (1, SH_G),
    'ae_w3T': (D_G, 1), 'ae_b3': (1, 1),
    'cr_w1T_s': (D_G, SH_G), 'cr_b1_s': (1, SH_G),
    'cr_w2T_s': (D_G, SH_G), 'cr_b2_s': (1, SH_G),
    'cr_w3T': (D_G, 1), 'cr_b3': (1, 1),
}


def host_prep(hand, draw_pile, disc_pile, character, monster, energy, params,
              hand_size, valid_action_mask):
    """Build the 8 per-core device input dicts from the full inputs."""
    p = {k: np.asarray(v, np.float32) for k, v in params.items()}
    f32 = np.float32

    cards = np.concatenate([np.asarray(hand, f32), np.asarray(draw_pile, f32),
                            np.asarray(disc_pile, f32)], 0)
    cardsT = np.ascontiguousarray(cards.T)

    maskfull = np.full((S, S), NEG, f32)
    maskfull[:S_HAND, :int(hand_size)] = 0.0
    maskfull[S_HAND:S_HAND + S_DRAW, S_HAND:S_HAND + S_DRAW] = 0.0
    maskfull[S_HAND + S_DRAW:, S_HAND + S_DRAW:] = 0.0

    sel = np.zeros((S, 6), f32)
    sel[:S_HAND, 0] = 1.0 / S_HAND
    sel[:S_HAND, 1] = 1.0
    sel[S_HAND:S_HAND + S_DRAW, 2] = 1.0 / S_DRAW
    sel[S_HAND:S_HAND + S_DRAW, 3] = 1.0
    sel[S_HAND + S_DRAW:, 4] = 1.0 / S_DISC
    sel[S_HAND + S_DRAW:, 5] = 1.0

    maskbias22 = np.where(np.asarray(valid_action_mask) == 0, NEG, 0.0).astype(f32)

    # fold the lnac affine into the ac head's first layer:
    #   W1 @ (g*xn + b) + b1 == (W1*g) @ xn + (b1 + W1 @ b)
    ac_w1g = p['ac_w1'] * p['lnac_g'][None, :]
    ac_b1f = p['ac_b1'] + p['ac_w1'] @ p['lnac_b']

    shared = dict(
        cardsT=cardsT, sel=sel, maskfull=maskfull, maskbias22=maskbias22[None, :],
        otherT=np.concatenate([np.asarray(character, f32), np.asarray(monster, f32),
                               np.asarray(energy, f32)])[:, None],
        emb_wT=np.ascontiguousarray(p['emb_card_w'].T), emb_b=p['emb_card_b'][None, :],
        inproj_wT=np.ascontiguousarray(p['in_proj_w'].T), inproj_b=p['in_proj_b'][None, :],
        outproj_wT=np.ascontiguousarray(p['out_proj_w'].T), outproj_b=p['out_proj_b'][None, :],
        ln1_g=p['ln1_g'], ln1_b=p['ln1_b'],
        ff1_wT=np.ascontiguousarray(p['ff1_w'].T), ff1_b=p['ff1_b'][None, :],
        ff2_wT=np.ascontiguousarray(p['ff2_w'].T), ff2_b=p['ff2_b'][None, :],
        ln2_g=p['ln2_g'], ln2_b=p['ln2_b'],
        other_w1T=np.ascontiguousarray(p['other_w1'].T), other_b1=p['other_b1'][None, :],
        other_w2T=np.ascontiguousarray(p['other_w2'].T), other_b2=p['other_b2'][None, :],
        lng_g=p['lng_g'], lng_b=p['lng_b'],
    )
    w1 = {'ac': ac_w1g, 'am': p['am_w1'], 'ae': p['ae_w1'], 'cr': p['cr_w1']}
    b1 = {'ac': ac_b1f, 'am': p['am_b1'], 'ae': p['ae_b1'], 'cr': p['cr_b1']}
    for n in ('ac', 'am', 'ae', 'cr'):
        shared[n + '_w3T'] = np.ascontiguousarray(p[n + '_w3'].T)
        shared[n + '_b3'] = p[n + '_b3'][None, :]

    per_core = []
    for c in range(NCORES):
        m = dict(shared)
        for n, sh in (('ac', SH_AC), ('am', SH_G), ('ae', SH_G), ('cr', SH_G)):
            sl = slice(c * sh, (c + 1) * sh)
            m[n + '_w1T_s'] = np.ascontiguousarray(w1[n].T[:, sl])
            m[n + '_b1_s'] = b1[n][None, sl]
            m[n + '_w2T_s'] = np.ascontiguousarray(p[n + '_w2'].T[:, sl])
            m[n + '_b2_s'] = p[n + '_b2'][None, sl]
        per_core.append(m)
    return per_core


def emit(ctx: ExitStack, tc: tile.TileContext, ins, probs_out, value_out,
         n_cores=NCORES):
    nc = tc.nc
    RG = [list(range(n_cores))]

    # ---- persistent pools (live for the whole kernel) ----
    consts = ctx.enter_context(tc.tile_pool(name="consts", bufs=1))
    keep = ctx.enter_context(tc.tile_pool(name="keep", bufs=1))
    tmp = ctx.enter_context(tc.tile_pool(name="tmp", bufs=2))
    wac = ctx.enter_context(tc.tile_pool(name="wac", bufs=14))
    wg = ctx.enter_context(tc.tile_pool(name="wg", bufs=14))
    ps = ctx.enter_context(tc.tile_pool(name="ps", bufs=3, space="PSUM"))
    pst = ctx.enter_context(tc.tile_pool(name="pst", bufs=2, space="PSUM"))
    psv = ctx.enter_context(tc.tile_pool(name="psv", bufs=2, space="PSUM"))
    dram = ctx.enter_context(tc.tile_pool(name="dram", bufs=1, space="DRAM"))

    ident = consts.tile([128, 128], F32)
    make_identity(nc, ident)
    ones = consts.tile([1, 64], F32)
    nc.vector.memset(ones, 1.0)
    eps55 = consts.tile([S, 1], F32)
    nc.vector.memset(eps55, EPS)

    def sb(name, pool, tag=None, bufs=None):
        """Load an input into SBUF with its natural (row) shape."""
        shp = list(INPUT_SPEC[name])
        if len(shp) == 1:
            shp = [1] + shp
        t = pool.tile(shp, F32, tag=tag or name, name=name + "_sb", bufs=bufs)
        src = ins[name]
        if len(INPUT_SPEC[name]) == 1:
            src = src.rearrange("(o n) -> o n", o=1)
        nc.sync.dma_start(out=t, in_=src)
        return t

    def trans(in_ap, P, Fr):
        """PE-transpose a [P, Fr] SBUF AP -> [Fr, P] PSUM tile."""
        pt = pst.tile([Fr, P], F32, tag="ptr", name="ptr")
        nc.tensor.transpose(pt, in_ap, ident[0:P, 0:P])
        return pt

    def ln_normalize(x, P, Dm):
        """In-place (x - mean) / sqrt(var + eps) over the free axis."""
        nsub = Dm // 512
        stats = tmp.tile([P, nsub, 6], F32, tag="lnstats", name="lnstats")
        xs = x.rearrange("p (n d) -> p n d", n=nsub) if nsub > 1 else x
        for i in range(nsub):
            nc.vector.bn_stats(out=stats[:, i, :],
                               in_=(xs[:, i, :] if nsub > 1 else xs))
        mv = tmp.tile([P, 2], F32, tag="lnmv", name="lnmv")
        nc.vector.bn_aggr(out=mv, in_=stats)
        nc.scalar.activation(out=mv[:, 1:2], in_=mv[:, 1:2], func=AF.Sqrt,
                             bias=eps55[0:P], scale=1.0)
        nc.vector.reciprocal(out=mv[:, 1:2], in_=mv[:, 1:2])
        nc.vector.tensor_scalar(out=x, in0=x, scalar1=mv[:, 0:1], scalar2=mv[:, 1:2],
                                op0=ALU.subtract, op1=ALU.mult)

    def ln_affine_chunked(x, P, Dm, g_name, b_name, pool):
        """x = x*g + b with g/b 1-D [Dm] inputs, applied in 512-wide chunks
        via partition-broadcast DMA loads (small SBUF footprint)."""
        for j in range(Dm // 512):
            gt = pool.tile([P, 512], F32, tag="lnaff", name="lnaff_g", bufs=4)
            bt = pool.tile([P, 512], F32, tag="lnaff", name="lnaff_b", bufs=4)
            for t, nm in ((gt, g_name), (bt, b_name)):
                src = ins[nm][j * 512:(j + 1) * 512]
                ap = bass.AP(tensor=src.tensor, offset=src.offset,
                             ap=[[0, P]] + src.ap)
                nc.gpsimd.dma_start(out=t, in_=ap)
            sl = x[:, j * 512:(j + 1) * 512]
            nc.vector.tensor_mul(out=sl, in0=sl, in1=gt)
            nc.vector.tensor_add(out=sl, in0=sl, in1=bt)

    # persistent cross-phase tensors
    acT = keep.tile([128, 36 * S_HAND], F32)       # ac_in^T K-tiles
    xgT = keep.tile([128, 32], F32)                # x_global^T K-tiles
    xg_d = dram.tile([1, D_G], F32)

    # ================= trunk (replicated, scoped pools) =================
    with tc.tile_pool(name="trunk", bufs=1) as trunk, \
         tc.tile_pool(name="ttmp", bufs=2) as ttmp, \
         tc.tile_pool(name="wtrunk", bufs=2) as wtrunk:

        def act55(name):
            return trunk.tile([S, D], F32, tag="act55", name=name, bufs=3)

        cardsT = sb('cardsT', trunk)
        emb_wT = sb('emb_wT', trunk)
        emb_b = sb('emb_b', trunk, tag="brow", bufs=3)
        px = ps.tile([S, D], F32, tag="ps", name="px")
        nc.tensor.matmul(px, lhsT=cardsT, rhs=emb_wT, start=True, stop=False)
        nc.tensor.matmul(px, lhsT=ones[0:1, 0:S], rhs=emb_b, start=False, stop=True)
        X0 = act55("X0")
        nc.vector.tensor_copy(out=X0, in_=px)

        X0T = trunk.tile([128, 4 * S], F32)
        for k in range(4):
            pt = trans(X0[:, k * 128:(k + 1) * 128], S, 128)
            nc.vector.tensor_copy(out=X0T[:, k * S:(k + 1) * S], in_=pt)

        # qkv: k-outer loop so only 2 weight tiles are in flight
        inproj_b = sb('inproj_b', trunk)
        pqkv = [ps.tile([S, D], F32, tag="ps", name=f"pqkv{qi}") for qi in range(3)]
        for k in range(4):
            wt = wtrunk.tile([128, 3 * D], F32, tag="tw", name="tw")
            nc.sync.dma_start(out=wt, in_=ins['inproj_wT'][k * 128:(k + 1) * 128, :])
            for qi in range(3):
                nc.tensor.matmul(pqkv[qi], lhsT=X0T[:, k * S:(k + 1) * S],
                                 rhs=wt[:, qi * D:(qi + 1) * D],
                                 start=(k == 0), stop=False)
        QKV = []
        for qi in range(3):
            nc.tensor.matmul(pqkv[qi], lhsT=ones[0:1, 0:S],
                             rhs=inproj_b[:, qi * D:(qi + 1) * D],
                             start=False, stop=True)
            t = trunk.tile([S, D], F32, tag=f"qkv{qi}", name=f"qkv{qi}")
            nc.vector.tensor_copy(out=t, in_=pqkv[qi])
            QKV.append(t)
        Q, K, V = QKV

        maskfull = sb('maskfull', trunk)
        AOp = ps.tile([S, D], F32, tag="ao", name="AOp", bufs=1)
        for h in range(NH):
            hs = slice(h * HD, (h + 1) * HD)
            qT = ttmp.tile([HD, S], F32, tag="qT", name="qT")
            nc.vector.tensor_copy(out=qT, in_=trans(Q[:, hs], S, HD))
            kT = ttmp.tile([HD, S], F32, tag="kT", name="kT")
            nc.vector.tensor_copy(out=kT, in_=trans(K[:, hs], S, HD))
            psc = pst.tile([S, S], F32, tag="ptr", name="psc")
            nc.tensor.matmul(psc, lhsT=qT, rhs=kT, start=True, stop=True)
            sc = ttmp.tile([S, S], F32, tag="sc", name="sc")
            nc.scalar.activation(out=sc, in_=psc, func=AF.Copy, scale=0.125)
            nc.vector.tensor_add(out=sc, in0=sc, in1=maskfull)
            mx = ttmp.tile([S, 1], F32, tag="mx", name="mx")
            nc.vector.reduce_max(out=mx, in_=sc, axis=AXX.X)
            nc.vector.tensor_scalar_sub(out=sc, in0=sc, scalar1=mx)
            nc.scalar.activation(out=sc, in_=sc, func=AF.Exp)
            sm = ttmp.tile([S, 1], F32, tag="sm", name="sm")
            nc.vector.reduce_sum(out=sm, in_=sc, axis=AXX.X)
            nc.vector.reciprocal(out=sm, in_=sm)
            nc.vector.tensor_scalar_mul(out=sc, in0=sc, scalar1=sm)
            aT = ttmp.tile([S, S], F32, tag="aT", name="aT")
            nc.vector.tensor_copy(out=aT, in_=trans(sc, S, S))
            nc.tensor.matmul(AOp[:, hs], lhsT=aT, rhs=V[:, hs], start=True, stop=True)
        AO = act55("AO")
        nc.vector.tensor_copy(out=AO, in_=AOp)

        def ln_bcast(x, P, g_name, b_name):
            ln_normalize(x, P, D)
            ln_affine_chunked(x, P, D, g_name, b_name, ttmp)

        def mlp512(x_sb, wT_name, b_name, out_name, extra_add=None):
            xT = trunk.tile([128, 4 * S], F32, tag="xT", name=wT_name + "_xT", bufs=2)
            for k in range(4):
                pt = trans(x_sb[:, k * 128:(k + 1) * 128], S, 128)
                nc.vector.tensor_copy(out=xT[:, k * S:(k + 1) * S], in_=pt)
            b_sb = sb(b_name, trunk, tag="brow", bufs=3)
            po = ps.tile([S, D], F32, tag="ps", name=wT_name + "_po")
            for k in range(4):
                wt = wtrunk.tile([128, D], F32, tag="tw2", name="tw2")
                nc.sync.dma_start(out=wt, in_=ins[wT_name][k * 128:(k + 1) * 128, :])
                nc.tensor.matmul(po, lhsT=xT[:, k * S:(k + 1) * S], rhs=wt,
                                 start=(k == 0), stop=False)
            nc.tensor.matmul(po, lhsT=ones[0:1, 0:S], rhs=b_sb, start=False, stop=True)
            o = act55(out_name)
            if extra_add is not None:
                nc.vector.tensor_add(out=o, in0=po, in1=extra_add)
            else:
                nc.vector.tensor_copy(out=o, in_=po)
            return o

        X1 = mlp512(AO, 'outproj_wT', 'outproj_b', "X1", extra_add=X0)
        ln_bcast(X1, S, 'ln1_g', 'ln1_b')
        Rf = mlp512(X1, 'ff1_wT', 'ff1_b', "Rf")
        nc.scalar.activation(out=Rf, in_=Rf, func=AF.Relu)
        X2 = mlp512(Rf, 'ff2_wT', 'ff2_b', "X2", extra_add=X1)
        ln_bcast(X2, S, 'ln2_g', 'ln2_b')

        # pile/hand aggregates AGG [6,512] -> DRAM (flattened to one row)
        sel = sb('sel', trunk)
        pagg = psv.tile([6, D], F32, tag="psv", name="pagg")
        nc.tensor.matmul(pagg, lhsT=sel, rhs=X2, start=True, stop=True)
        agg_d = dram.tile([1, 6 * D], F32)
        agg_sb = ttmp.tile([6, D], F32, tag="aggsb", name="aggsb")
        nc.vector.tensor_copy(out=agg_sb, in_=pagg)
        nc.sync.dma_start(out=agg_d[0:1, :].rearrange("o (r d) -> (o r) d", r=6),
                          in_=agg_sb)

        # other-MLP
        otherT = sb('otherT', trunk)
        ob1 = sb('other_b1', trunk)
        ow1 = wtrunk.tile([120, 2 * D], F32, tag="ow1", name="ow1")
        nc.sync.dma_start(out=ow1, in_=ins['other_w1T'])
        O1 = trunk.tile([1, 2 * D], F32)
        for half in range(2):
            pv = psv.tile([1, D], F32, tag="psv", name="po1")
            nc.tensor.matmul(pv, lhsT=otherT, rhs=ow1[:, half * D:(half + 1) * D],
                             start=True, stop=False)
            nc.tensor.matmul(pv, lhsT=ones[0:1, 0:1],
                             rhs=ob1[:, half * D:(half + 1) * D],
                             start=False, stop=True)
            nc.scalar.activation(out=O1[:, half * D:(half + 1) * D], in_=pv,
                                 func=AF.Relu)
        o1_d = dram.tile([1, 2 * D], F32)
        nc.sync.dma_start(out=o1_d, in_=O1)
        o1n = ttmp.tile([8, 128], F32, tag="o1n", name="o1n")
        nc.sync.dma_start(out=o1n,
                          in_=o1_d[0:1, :].rearrange("o (r p) -> (o r) p", p=128))
        O1T = trunk.tile([128, 8], F32)
        nc.vector.tensor_copy(out=O1T, in_=trans(o1n, 8, 128))
        ob2 = sb('other_b2', trunk)
        O2 = trunk.tile([1, 2 * D], F32)
        for half in range(2):
            pv = psv.tile([1, D], F32, tag="psv", name="po2")
            for k in range(8):
                wt = wtrunk.tile([128, D], F32, tag="ow2", name="ow2")
                nc.sync.dma_start(out=wt,
                                  in_=ins['other_w2T'][k * 128:(k + 1) * 128,
                                                       half * D:(half + 1) * D])
                nc.tensor.matmul(pv, lhsT=O1T[:, k:k + 1], rhs=wt,
                                 start=(k == 0), stop=False)
            nc.tensor.matmul(pv, lhsT=ones[0:1, 0:1],
                             rhs=ob2[:, half * D:(half + 1) * D],
                             start=False, stop=True)
            nc.scalar.activation(out=O2[:, half * D:(half + 1) * D], in_=pv,
                                 func=AF.Relu)

        # x_global [1,4096]: concat + LN (affine chunked elementwise, P=1)
        XG = trunk.tile([1, D_G], F32, tag="bigx", name="XG", bufs=1)
        nc.sync.dma_start(out=XG[0:1, 0:6 * D], in_=agg_d)
        nc.vector.tensor_copy(out=XG[0:1, 6 * D:8 * D], in_=O2)
        ln_normalize(XG, 1, D_G)
        ln_affine_chunked(XG, 1, D_G, 'lng_g', 'lng_b', ttmp)
        nc.sync.dma_start(out=xg_d, in_=XG)

        # ac_in [10,4608] = bcast(xg) || x_hand, then lnac NORMALIZE only
        # (affine folded into ac_w1/ac_b1 on host)
        ACIN = trunk.tile([S_HAND, D_AC], F32, tag="bigx", name="ACIN", bufs=1)
        xgap = xg_d[:]
        xg_bc = bass.AP(tensor=xgap.tensor, offset=xgap.offset,
                        ap=[[0, S_HAND]] + xgap.ap[-1:])
        nc.gpsimd.dma_start(out=ACIN[:, 0:D_G], in_=xg_bc)
        nc.vector.tensor_copy(out=ACIN[:, D_G:D_AC], in_=X2[0:S_HAND, :])
        ln_normalize(ACIN, S_HAND, D_AC)

        # transposed K-tiles for the head matmuls (into persistent tiles)
        xgn = ttmp.tile([32, 128], F32, tag="xgn", name="xgn")
        nc.sync.dma_start(out=xgn,
                          in_=xg_d[0:1, :].rearrange("o (r p) -> (o r) p", p=128))
        nc.vector.tensor_copy(out=xgT, in_=trans(xgn, 32, 128))
        for k in range(36):
            pt = trans(ACIN[:, k * 128:(k + 1) * 128], S_HAND, 128)
            nc.vector.tensor_copy(out=acT[:, k * S_HAND:(k + 1) * S_HAND], in_=pt)
    # ================= end trunk scope =================

    bounce1 = dram.tile([1, BN], F32)
    gout1 = dram.tile([n_cores, BN], F32)

    def head_ac_layer(lhsT_tiles, w_name, b_name, bounce, ktiles):
        b_sb = sb(b_name, keep)
        pa = ps.tile([S_HAND, 512], F32, tag="ps", name=w_name + "_pa")
        pb = psv.tile([S_HAND, 64], F32, tag="psv", name=w_name + "_pb")
        for i, (row, sz) in enumerate(ktiles):
            wt = wac.tile([128, SH_AC], F32, tag="wac", name="wac")
            nc.sync.dma_start(out=wt[0:sz, :], in_=ins[w_name][row:row + sz, :])
            nc.tensor.matmul(pa, lhsT=lhsT_tiles(i, sz), rhs=wt[0:sz, 0:512],
                             start=(i == 0), stop=False)
            nc.tensor.matmul(pb, lhsT=lhsT_tiles(i, sz), rhs=wt[0:sz, 512:SH_AC],
                             start=(i == 0), stop=False)
        nc.tensor.matmul(pa, lhsT=ones[0:1, 0:S_HAND], rhs=b_sb[:, 0:512],
                         start=False, stop=True)
        nc.tensor.matmul(pb, lhsT=ones[0:1, 0:S_HAND], rhs=b_sb[:, 512:SH_AC],
                         start=False, stop=True)
        h = tmp.tile([S_HAND, SH_AC], F32, tag="h_ac", name="h_ac")
        nc.scalar.activation(out=h[:, 0:512], in_=pa, func=AF.Relu)
        nc.scalar.activation(out=h[:, 512:SH_AC], in_=pb, func=AF.Relu)
        nc.sync.dma_start(
            out=bounce[0:1, OFF_AC:OFF_AC + S_HAND * SH_AC].rearrange(
                "o (t d) -> (o t) d", t=S_HAND),
            in_=h)

    def head_g_layer(lhsT_tiles, w_name, b_name, bounce, off, nk):
        b_sb = sb(b_name, keep)
        pv = psv.tile([1, SH_G], F32, tag="psv", name=w_name + "_pv")
        for k in range(nk):
            wt = wg.tile([128, SH_G], F32, tag="wg", name="wg")
            nc.sync.dma_start(out=wt, in_=ins[w_name][k * 128:(k + 1) * 128, :])
            nc.tensor.matmul(pv, lhsT=lhsT_tiles(k), rhs=wt, start=(k == 0),
                             stop=False)
        nc.tensor.matmul(pv, lhsT=ones[0:1, 0:1], rhs=b_sb, start=False, stop=True)
        h = tmp.tile([1, SH_G], F32, tag="h_g", name="h_g")
        nc.scalar.activation(out=h, in_=pv, func=AF.Relu)
        nc.sync.dma_start(out=bounce[0:1, off:off + SH_G], in_=h)

    head_ac_layer(lambda i, sz: acT[:, i * S_HAND:(i + 1) * S_HAND], 'ac_w1T_s',
                  'ac_b1_s', bounce1, [(k * 128, 128) for k in range(36)])
    head_g_layer(lambda k: xgT[:, k:k + 1], 'am_w1T_s', 'am_b1_s', bounce1, OFF_AM, 32)
    head_g_layer(lambda k: xgT[:, k:k + 1], 'ae_w1T_s', 'ae_b1_s', bounce1, OFF_AE, 32)
    head_g_layer(lambda k: xgT[:, k:k + 1], 'cr_w1T_s', 'cr_b1_s', bounce1, OFF_CR, 32)

    nc.gpsimd.collective_compute(
        "AllGather", ALU.bypass, replica_groups=RG,
        ins=[bounce1[:].opt()], outs=[gout1[:].opt()])

    def load_gathered(gout, tag):
        """Load + transpose the gathered h into lhsT K-tiles."""
        acK = keep.tile([128, len(AC_TILES) * S_HAND], F32, tag=tag + "acK",
                        name=tag + "acK")
        for c in range(n_cores):
            yc = tmp.tile([S_HAND, SH_AC], F32, tag="yc", name=tag + "yc")
            nc.sync.dma_start(
                out=yc,
                in_=gout[c:c + 1, OFF_AC:OFF_AC + S_HAND * SH_AC].rearrange(
                    "o (t d) -> (o t) d", t=S_HAND))
            for j, (off, sz) in enumerate(AC_BLK):
                i = c * len(AC_BLK) + j
                pt = trans(yc[:, off:off + sz], S_HAND, sz)
                nc.vector.tensor_copy(out=acK[0:sz, i * S_HAND:(i + 1) * S_HAND],
                                      in_=pt)
        gK = {}
        for nm, off in (('am', OFF_AM), ('ae', OFF_AE), ('cr', OFF_CR)):
            gn = tmp.tile([32, 128], F32, tag="gn", name=tag + nm + "gn")
            for c in range(n_cores):
                nc.sync.dma_start(
                    out=gn[c * 4:(c + 1) * 4, :],
                    in_=gout[c:c + 1, off:off + SH_G].rearrange(
                        "o (r p) -> (o r) p", p=128))
            t = keep.tile([128, 32], F32, tag=tag + nm, name=tag + nm)
            nc.vector.tensor_copy(out=t, in_=trans(gn, 32, 128))
            gK[nm] = t
        return acK, gK

    h1acK, h1gK = load_gathered(gout1, "g1")

    bounce2 = dram.tile([1, BN], F32)
    gout2 = dram.tile([n_cores, BN], F32)
    head_ac_layer(lambda i, sz: h1acK[0:sz, i * S_HAND:(i + 1) * S_HAND], 'ac_w2T_s',
                  'ac_b2_s', bounce2, AC_TILES)
    head_g_layer(lambda k: h1gK['am'][:, k:k + 1], 'am_w2T_s', 'am_b2_s', bounce2,
                 OFF_AM, 32)
    head_g_layer(lambda k: h1gK['ae'][:, k:k + 1], 'ae_w2T_s', 'ae_b2_s', bounce2,
                 OFF_AE, 32)
    head_g_layer(lambda k: h1gK['cr'][:, k:k + 1], 'cr_w2T_s', 'cr_b2_s', bounce2,
                 OFF_CR, 32)

    nc.gpsimd.collective_compute(
        "AllGather", ALU.bypass, replica_groups=RG,
        ins=[bounce2[:].opt()], outs=[gout2[:].opt()])

    h2acK, h2gK = load_gathered(gout2, "g2")

    # ---------------- W3 + tail (replicated) ----------------
    w3sb = keep.tile([128, len(AC_TILES), 2], F32)
    for i, (row, sz) in enumerate(AC_TILES):
        nc.sync.dma_start(out=w3sb[0:sz, i, :], in_=ins['ac_w3T'][row:row + sz, :])
    ac_b3 = sb('ac_b3', keep)
    plp = psv.tile([2, S_HAND], F32, tag="psv", name="plp")
    for i, (row, sz) in enumerate(AC_TILES):
        nc.tensor.matmul(plp, lhsT=w3sb[0:sz, i, :],
                         rhs=h2acK[0:sz, i * S_HAND:(i + 1) * S_HAND],
                         start=(i == 0), stop=False)
    nc.tensor.matmul(plp, lhsT=ac_b3, rhs=ones[0:1, 0:S_HAND], start=False, stop=True)
    lpT_sb = tmp.tile([2, S_HAND], F32, tag="lpT", name="lpT")
    nc.vector.tensor_copy(out=lpT_sb, in_=plp)
    lp_d = dram.tile([1, 2 * S_HAND], F32)
    nc.sync.dma_start(out=lp_d[0:1, :].rearrange("o (a b) -> (o a) b", a=2),
                      in_=lpT_sb)

    scal = {}
    for nm in ('am', 'ae', 'cr'):
        w3g = keep.tile([128, 32], F32, tag=nm + "w3", name=nm + "w3")
        nc.sync.dma_start(out=w3g,
                          in_=ins[nm + '_w3T'].rearrange("(k p) o -> p (k o)", p=128))
        b3 = sb(nm + '_b3', keep)
        pv = psv.tile([1, 1], F32, tag="psv", name=nm + "_p3")
        for k in range(32):
            nc.tensor.matmul(pv, lhsT=w3g[:, k:k + 1], rhs=h2gK[nm][:, k:k + 1],
                             start=(k == 0), stop=False)
        nc.tensor.matmul(pv, lhsT=b3, rhs=ones[0:1, 0:1], start=False, stop=True)
        t = keep.tile([1, 1], F32, tag=nm + "res", name=nm + "res")
        nc.vector.tensor_copy(out=t, in_=pv)
        scal[nm] = t

    logits = keep.tile([1, 22], F32)
    nc.sync.dma_start(out=logits[0:1, 0:20], in_=lp_d)
    nc.vector.tensor_copy(out=logits[0:1, 20:21], in_=scal['am'])
    nc.vector.tensor_copy(out=logits[0:1, 21:22], in_=scal['ae'])
    mb = sb('maskbias22', keep)
    nc.vector.tensor_add(out=logits, in0=logits, in1=mb)
    mx = keep.tile([1, 1], F32, name="fmx")
    nc.vector.reduce_max(out=mx, in_=logits, axis=AXX.X)
    nc.vector.tensor_scalar_sub(out=logits, in0=logits, scalar1=mx)
    nc.scalar.activation(out=logits, in_=logits, func=AF.Exp)
    sm = keep.tile([1, 1], F32, name="fsm")
    nc.vector.reduce_sum(out=sm, in_=logits, axis=AXX.X)
    nc.vector.reciprocal(out=sm, in_=sm)
    nc.vector.tensor_scalar_mul(out=logits, in0=logits, scalar1=sm)
    nc.sync.dma_start(out=probs_out, in_=logits)
    nc.sync.dma_start(out=value_out, in_=scal['cr'])


def build_program(n_cores=NCORES):
    nc = bacc.Bacc("TRN2", target_bir_lowering=False, debug=False,
                   num_devices=n_cores)
    ins = {}
    for name, shape in INPUT_SPEC.items():
        ins[name] = nc.dram_tensor(name, list(shape), F32, kind="ExternalInput").ap()
    probs = nc.dram_tensor("probs", [1, 22], F32, kind="ExternalOutput").ap()
    value = nc.dram_tensor("value", [1, 1], F32, kind="ExternalOutput").ap()
    with tile.TileContext(nc) as tc:
        with ExitStack() as ctx:
            emit(ctx, tc, ins, probs, value, n_cores=n_cores)
    nc.compile()
    return nc


_PROG = {}


def _get_program(n_cores=NCORES):
    if n_cores not in _PROG:
        _PROG[n_cores] = build_program(n_cores)
    return _PROG[n_cores]


def kernel(**inputs):
    per_core = host_prep(**inputs)
    nc = _get_program(NCORES)
    res = run_bass_kernel_spmd(nc, per_core, core_ids=list(range(NCORES)))
    out = res.results[0]
    probs = np.asarray(out['probs'], np.float32).reshape(22)
    value = np.asarray(out['value'], np.float32).reshape(1)
    return probs, value


# revision 10
# speedup vs baseline: 1.0752x; 1.0752x over previous
"""Trainium2 Bass kernel for nn_ActorCritic forward (8-core tensor-parallel).

Strategy (memory-bound problem — ~573MB of head weights dominate):
  - Trunk (embedding + one shared transformer block over the 55 concatenated
    hand/draw/disc tokens with block-diagonal attention masking, other-MLP,
    global feature assembly + layernorms) is small and replicated on all
    8 cores.
  - Head MLPs (ac: 4608^2 x2, am/ae/cr: 4096^2 x2) are column-parallel
    across the 8 cores: each core computes a 1/8 slice of
    h1 = relu(W1 x + b1), one AllGather rebuilds h1 everywhere, each core
    computes its h2 slice, then each core contracts its h2 slice with its
    W3 column-shard and a tiny AllReduce (24 floats) sums the partial
    logits/value. The masked softmax tail runs redundantly on every core.
  - Weights ride the PE's MOVING operand (rhs) so weight bytes stream
    HBM->SBUF->PE at DMA rate; weight DMAs are batched 4 K-tiles per
    dma_start (~1MB each) and alternate between the two HWDGE rings
    (sync + scalar engines) to amortize issue cost.
  - The lnac affine is folded into ac W1 on the host; head-layer biases are
    rank-1 matmul accumulates; W3 biases are folded into the tail mask add.
"""

import numpy as np
from contextlib import ExitStack

import concourse.bass as bass
import concourse.bacc as bacc
import concourse.tile as tile
from concourse import mybir
from concourse.bass_utils import run_bass_kernel_spmd
from concourse.masks import make_identity

F32 = mybir.dt.float32
AF = mybir.ActivationFunctionType
ALU = mybir.AluOpType
AXX = mybir.AxisListType

NCORES = 8
D = 512
S_HAND, S_DRAW, S_DISC = 10, 30, 15
S = S_HAND + S_DRAW + S_DISC          # 55
NH, HD = 8, 64
EPS = 1e-5
D_AC = 9 * D                           # 4608
D_G = 8 * D                            # 4096
SH_AC = D_AC // NCORES                 # 576
SH_G = D_G // NCORES                   # 512
NEG = -1e30
BN = 7296                              # AG1 bounce: 10*576 + 3*512
OFF_AC, OFF_AM, OFF_AE, OFF_CR = 0, 5760, 6272, 6784
AC_BLK = [(j * 128, min(128, SH_AC - j * 128)) for j in range((SH_AC + 127) // 128)]

INPUT_SPEC = {
    'cardsT': (128, S), 'sel': (S, 6), 'maskfull': (S, S), 'maskbias22': (1, 22),
    'otherT': (120, 1), 'cr_b3': (1, 1),
    'emb_wT': (128, D), 'emb_b': (1, D),
    'inproj_wT': (D, 3 * D), 'inproj_b': (1, 3 * D),
    'outproj_wT': (D, D), 'outproj_b': (1, D),
    'ln1_g': (D,), 'ln1_b': (D,),
    'ff1_wT': (D, D), 'ff1_b': (1, D),
    'ff2_wT': (D, D), 'ff2_b': (1, D),
    'ln2_g': (D,), 'ln2_b': (D,),
    'other_w1T': (120, 2 * D), 'other_b1': (1, 2 * D),
    'other_w2T': (2 * D, 2 * D), 'other_b2': (1, 2 * D),
    'lng_g': (D_G,), 'lng_b': (D_G,),
    'ac_w1T_s': (D_AC, SH_AC), 'ac_b1_s': (1, SH_AC),
    'ac_w2T_s': (D_AC, SH_AC), 'ac_b2_s': (1, SH_AC),
    'ac_w3T_s': (SH_AC, 2),
    'am_w1T_s': (D_G, SH_G), 'am_b1_s': (1, SH_G),
    'am_w2T_s': (D_G, SH_G), 'am_b2_s': (1, SH_G),
    'am_w3_s': (1, SH_G),
    'ae_w1T_s': (D_G, SH_G), 'ae_b1_s': (1, SH_G),
    'ae_w2T_s': (D_G, SH_G), 'ae_b2_s': (1, SH_G),
    'ae_w3_s': (1, SH_G),
    'cr_w1T_s': (D_G, SH_G), 'cr_b1_s': (1, SH_G),
    'cr_w2T_s': (D_G, SH_G), 'cr_b2_s': (1, SH_G),
    'cr_w3_s': (1, SH_G),
}


def host_prep(hand, draw_pile, disc_pile, character, monster, energy, params,
              hand_size, valid_action_mask):
    """Build the 8 per-core device input dicts from the full inputs."""
    p = {k: np.asarray(v, np.float32) for k, v in params.items()}
    f32 = np.float32

    cards = np.concatenate([np.asarray(hand, f32), np.asarray(draw_pile, f32),
                            np.asarray(disc_pile, f32)], 0)
    cardsT = np.ascontiguousarray(cards.T)

    maskfull = np.full((S, S), NEG, f32)
    maskfull[:S_HAND, :int(hand_size)] = 0.0
    maskfull[S_HAND:S_HAND + S_DRAW, S_HAND:S_HAND + S_DRAW] = 0.0
    maskfull[S_HAND + S_DRAW:, S_HAND + S_DRAW:] = 0.0

    sel = np.zeros((S, 6), f32)
    sel[:S_HAND, 0] = 1.0 / S_HAND
    sel[:S_HAND, 1] = 1.0
    sel[S_HAND:S_HAND + S_DRAW, 2] = 1.0 / S_DRAW
    sel[S_HAND:S_HAND + S_DRAW, 3] = 1.0
    sel[S_HAND + S_DRAW:, 4] = 1.0 / S_DISC
    sel[S_HAND + S_DRAW:, 5] = 1.0

    # W3 biases are folded into the mask-add before the softmax
    maskbias22 = np.where(np.asarray(valid_action_mask) == 0, NEG, 0.0).astype(f32)
    maskbias22[0:10] += p['ac_b3'][0]
    maskbias22[10:20] += p['ac_b3'][1]
    maskbias22[20] += p['am_b3'][0]
    maskbias22[21] += p['ae_b3'][0]

    # fold the lnac affine into the ac head's first layer:
    #   W1 @ (g*xn + b) + b1 == (W1*g) @ xn + (b1 + W1 @ b)
    ac_w1g = p['ac_w1'] * p['lnac_g'][None, :]
    ac_b1f = p['ac_b1'] + p['ac_w1'] @ p['lnac_b']

    shared = dict(
        cardsT=cardsT, sel=sel, maskfull=maskfull, maskbias22=maskbias22[None, :],
        otherT=np.concatenate([np.asarray(character, f32), np.asarray(monster, f32),
                               np.asarray(energy, f32)])[:, None],
        cr_b3=p['cr_b3'][None, :],
        emb_wT=np.ascontiguousarray(p['emb_card_w'].T), emb_b=p['emb_card_b'][None, :],
        inproj_wT=np.ascontiguousarray(p['in_proj_w'].T), inproj_b=p['in_proj_b'][None, :],
        outproj_wT=np.ascontiguousarray(p['out_proj_w'].T), outproj_b=p['out_proj_b'][None, :],
        ln1_g=p['ln1_g'], ln1_b=p['ln1_b'],
        ff1_wT=np.ascontiguousarray(p['ff1_w'].T), ff1_b=p['ff1_b'][None, :],
        ff2_wT=np.ascontiguousarray(p['ff2_w'].T), ff2_b=p['ff2_b'][None, :],
        ln2_g=p['ln2_g'], ln2_b=p['ln2_b'],
        other_w1T=np.ascontiguousarray(p['other_w1'].T), other_b1=p['other_b1'][None, :],
        other_w2T=np.ascontiguousarray(p['other_w2'].T), other_b2=p['other_b2'][None, :],
        lng_g=p['lng_g'], lng_b=p['lng_b'],
    )
    w1 = {'ac': ac_w1g, 'am': p['am_w1'], 'ae': p['ae_w1'], 'cr': p['cr_w1']}
    b1 = {'ac': ac_b1f, 'am': p['am_b1'], 'ae': p['ae_b1'], 'cr': p['cr_b1']}

    per_core = []
    for c in range(NCORES):
        m = dict(shared)
        for n, sh in (('ac', SH_AC), ('am', SH_G), ('ae', SH_G), ('cr', SH_G)):
            sl = slice(c * sh, (c + 1) * sh)
            m[n + '_w1T_s'] = np.ascontiguousarray(w1[n].T[:, sl])
            m[n + '_b1_s'] = b1[n][None, sl]
            m[n + '_w2T_s'] = np.ascontiguousarray(p[n + '_w2'].T[:, sl])
            m[n + '_b2_s'] = p[n + '_b2'][None, sl]
            if n == 'ac':
                m['ac_w3T_s'] = np.ascontiguousarray(p['ac_w3'].T[sl, :])
            else:
                m[n + '_w3_s'] = p[n + '_w3'][:, sl].reshape(1, sh)
        per_core.append(m)
    return per_core


def emit(ctx: ExitStack, tc: tile.TileContext, ins, probs_out, value_out,
         n_cores=NCORES):
    nc = tc.nc
    RG = [list(range(n_cores))]

    consts = ctx.enter_context(tc.tile_pool(name="consts", bufs=1))
    keep = ctx.enter_context(tc.tile_pool(name="keep", bufs=1))
    tmp = ctx.enter_context(tc.tile_pool(name="tmp", bufs=2))
    ps = ctx.enter_context(tc.tile_pool(name="ps", bufs=3, space="PSUM"))
    pst = ctx.enter_context(tc.tile_pool(name="pst", bufs=2, space="PSUM"))
    psv = ctx.enter_context(tc.tile_pool(name="psv", bufs=2, space="PSUM"))
    dram = ctx.enter_context(tc.tile_pool(name="dram", bufs=1, space="DRAM"))

    ident = consts.tile([128, 128], F32)
    make_identity(nc, ident)
    ones = consts.tile([1, 64], F32)
    nc.vector.memset(ones, 1.0)
    eps55 = consts.tile([S, 1], F32)
    nc.vector.memset(eps55, EPS)

    # alternate big weight DMAs across the two HWDGE rings
    _dma_eng = [0]

    def wdma(out, in_):
        eng = nc.sync if _dma_eng[0] % 2 == 0 else nc.scalar
        _dma_eng[0] += 1
        eng.dma_start(out=out, in_=in_)

    def sb(name, pool, tag=None, bufs=None):
        shp = list(INPUT_SPEC[name])
        if len(shp) == 1:
            shp = [1] + shp
        t = pool.tile(shp, F32, tag=tag or name, name=name + "_sb", bufs=bufs)
        src = ins[name]
        if len(INPUT_SPEC[name]) == 1:
            src = src.rearrange("(o n) -> o n", o=1)
        nc.sync.dma_start(out=t, in_=src)
        return t

    def trans(in_ap, P, Fr):
        pt = pst.tile([Fr, P], F32, tag="ptr", name="ptr")
        nc.tensor.transpose(pt, in_ap, ident[0:P, 0:P])
        return pt

    def ln_normalize(x, P, Dm):
        nsub = Dm // 512
        stats = tmp.tile([P, nsub, 6], F32, tag="lnstats", name="lnstats")
        xs = x.rearrange("p (n d) -> p n d", n=nsub) if nsub > 1 else x
        for i in range(nsub):
            nc.vector.bn_stats(out=stats[:, i, :],
                               in_=(xs[:, i, :] if nsub > 1 else xs))
        mv = tmp.tile([P, 2], F32, tag="lnmv", name="lnmv")
        nc.vector.bn_aggr(out=mv, in_=stats)
        nc.scalar.activation(out=mv[:, 1:2], in_=mv[:, 1:2], func=AF.Sqrt,
                             bias=eps55[0:P], scale=1.0)
        nc.vector.reciprocal(out=mv[:, 1:2], in_=mv[:, 1:2])
        nc.vector.tensor_scalar(out=x, in0=x, scalar1=mv[:, 0:1], scalar2=mv[:, 1:2],
                                op0=ALU.subtract, op1=ALU.mult)

    def ln_affine_chunked(x, P, Dm, g_name, b_name, pool):
        for j in range(Dm // 512):
            gt = pool.tile([P, 512], F32, tag="lnaff", name="lnaff_g", bufs=3)
            bt = pool.tile([P, 512], F32, tag="lnaff", name="lnaff_b", bufs=3)
            for t, nm in ((gt, g_name), (bt, b_name)):
                src = ins[nm][j * 512:(j + 1) * 512]
                ap = bass.AP(tensor=src.tensor, offset=src.offset,
                             ap=[[0, P]] + src.ap)
                nc.gpsimd.dma_start(out=t, in_=ap)
            sl = x[:, j * 512:(j + 1) * 512]
            nc.vector.tensor_mul(out=sl, in0=sl, in1=gt)
            nc.vector.tensor_add(out=sl, in0=sl, in1=bt)

    acT = keep.tile([128, 36 * S_HAND], F32)       # ac_in^T K-tiles
    xgT = keep.tile([128, 32], F32)                # x_global^T K-tiles
    xg_d = dram.tile([1, D_G], F32)

    # ================= trunk + W1 (scoped pools) =================
    w1scope = ExitStack()
    wac = w1scope.enter_context(tc.tile_pool(name="wac", bufs=4))
    wg = w1scope.enter_context(tc.tile_pool(name="wg", bufs=3))
    with tc.tile_pool(name="trunk", bufs=1) as trunk, \
         tc.tile_pool(name="ttmp", bufs=2) as ttmp, \
         tc.tile_pool(name="wtrunk", bufs=2) as wtrunk:

        def act55(name):
            return trunk.tile([S, D], F32, tag="act55", name=name, bufs=3)

        cardsT = sb('cardsT', trunk)
        emb_wT = sb('emb_wT', trunk)
        emb_b = sb('emb_b', trunk, tag="brow", bufs=3)
        px = ps.tile([S, D], F32, tag="ps", name="px")
        nc.tensor.matmul(px, lhsT=cardsT, rhs=emb_wT, start=True, stop=False)
        nc.tensor.matmul(px, lhsT=ones[0:1, 0:S], rhs=emb_b, start=False, stop=True)
        X0 = act55("X0")
        nc.vector.tensor_copy(out=X0, in_=px)

        X0T = trunk.tile([128, 4 * S], F32)
        for k in range(4):
            pt = trans(X0[:, k * 128:(k + 1) * 128], S, 128)
            nc.vector.tensor_copy(out=X0T[:, k * S:(k + 1) * S], in_=pt)

        inproj_b = sb('inproj_b', trunk, tag='ipb', bufs=1)
        pqkv = [ps.tile([S, D], F32, tag="ps", name=f"pqkv{qi}") for qi in range(3)]
        for k in range(4):
            wt = wtrunk.tile([128, 3 * D], F32, tag="tw", name="tw")
            wdma(wt, ins['inproj_wT'][k * 128:(k + 1) * 128, :])
            for qi in range(3):
                nc.tensor.matmul(pqkv[qi], lhsT=X0T[:, k * S:(k + 1) * S],
                                 rhs=wt[:, qi * D:(qi + 1) * D],
                                 start=(k == 0), stop=False)
        QKV = []
        for qi in range(3):
            nc.tensor.matmul(pqkv[qi], lhsT=ones[0:1, 0:S],
                             rhs=inproj_b[:, qi * D:(qi + 1) * D],
                             start=False, stop=True)
            t = trunk.tile([S, D], F32, tag=f"qkv{qi}", name=f"qkv{qi}")
            nc.vector.tensor_copy(out=t, in_=pqkv[qi])
            QKV.append(t)
        Q, K, V = QKV

        # attention: full-Q/K transposes, per-head scores, batched softmax
        qTf = trunk.tile([128, 4 * S], F32)
        kTf = trunk.tile([128, 4 * S], F32)
        for k in range(4):
            nc.vector.tensor_copy(out=qTf[:, k * S:(k + 1) * S],
                                  in_=trans(Q[:, k * 128:(k + 1) * 128], S, 128))
            nc.vector.tensor_copy(out=kTf[:, k * S:(k + 1) * S],
                                  in_=trans(K[:, k * 128:(k + 1) * 128], S, 128))

        maskfull = sb('maskfull', trunk)
        SALL = trunk.tile([S, NH, S], F32)
        for h in range(NH):
            bp = (h % 2) * HD
            blk = slice((h // 2) * S, (h // 2 + 1) * S)
            psc = pst.tile([S, S], F32, tag="ptr", name="psc")
            nc.tensor.matmul(psc, lhsT=qTf[bp:bp + HD, blk],
                             rhs=kTf[bp:bp + HD, blk], start=True, stop=True)
            nc.scalar.activation(out=SALL[:, h, :], in_=psc, func=AF.Copy,
                                 scale=0.125)
        mfap = maskfull[:]
        nc.vector.tensor_add(
            out=SALL, in0=SALL,
            in1=bass.AP(tensor=mfap.tensor, offset=mfap.offset,
                        ap=[mfap.ap[0], [0, NH], mfap.ap[1]]))
        mx8 = ttmp.tile([S, NH], F32, tag="mx8", name="mx8")
        nc.vector.reduce_max(out=mx8, in_=SALL, axis=AXX.X)
        mxap = mx8[:]
        nc.vector.tensor_tensor(
            out=SALL, in0=SALL,
            in1=bass.AP(tensor=mxap.tensor, offset=mxap.offset,
                        ap=[mxap.ap[0], mxap.ap[1], [0, S]]),
            op=ALU.subtract)
        nc.scalar.activation(out=SALL, in_=SALL, func=AF.Exp)
        sm8 = ttmp.tile([S, NH], F32, tag="sm8", name="sm8")
        nc.vector.reduce_sum(out=sm8, in_=SALL, axis=AXX.X)
        nc.vector.reciprocal(out=sm8, in_=sm8)
        smap = sm8[:]
        nc.vector.tensor_tensor(
            out=SALL, in0=SALL,
            in1=bass.AP(tensor=smap.tensor, offset=smap.offset,
                        ap=[smap.ap[0], smap.ap[1], [0, S]]),
            op=ALU.mult)
        AOp = ps.tile([S, D], F32, tag="ao", name="AOp", bufs=1)
        for h in range(NH):
            hs = slice(h * HD, (h + 1) * HD)
            aT = ttmp.tile([S, S], F32, tag="aT", name="aT")
            nc.vector.tensor_copy(out=aT, in_=trans(SALL[:, h, :], S, S))
            nc.tensor.matmul(AOp[:, hs], lhsT=aT, rhs=V[:, hs], start=True,
                             stop=True)
        AO = act55("AO")
        nc.vector.tensor_copy(out=AO, in_=AOp)

        def ln_bcast(x, P, g_name, b_name):
            ln_normalize(x, P, D)
            ln_affine_chunked(x, P, D, g_name, b_name, ttmp)

        def mlp512(x_sb, wT_name, b_name, out_name, extra_add=None):
            xT = trunk.tile([128, 4 * S], F32, tag="xT", name=wT_name + "_xT",
                            bufs=2)
            for k in range(4):
                pt = trans(x_sb[:, k * 128:(k + 1) * 128], S, 128)
                nc.vector.tensor_copy(out=xT[:, k * S:(k + 1) * S], in_=pt)
            b_sb = sb(b_name, trunk, tag="brow", bufs=3)
            po = ps.tile([S, D], F32, tag="ps", name=wT_name + "_po")
            wt = wtrunk.tile([128, 4, D], F32, tag="tw2", name=wT_name + "_w")
            wdma(wt, ins[wT_name][:].rearrange("(a p) d -> p a d", p=128))
            for k in range(4):
                nc.tensor.matmul(po, lhsT=xT[:, k * S:(k + 1) * S], rhs=wt[:, k, :],
                                 start=(k == 0), stop=False)
            nc.tensor.matmul(po, lhsT=ones[0:1, 0:S], rhs=b_sb, start=False,
                             stop=True)
            o = act55(out_name)
            if extra_add is not None:
                nc.vector.tensor_add(out=o, in0=po, in1=extra_add)
            else:
                nc.vector.tensor_copy(out=o, in_=po)
            return o

        X1 = mlp512(AO, 'outproj_wT', 'outproj_b', "X1", extra_add=X0)
        ln_bcast(X1, S, 'ln1_g', 'ln1_b')
        Rf = mlp512(X1, 'ff1_wT', 'ff1_b', "Rf")
        nc.scalar.activation(out=Rf, in_=Rf, func=AF.Relu)
        X2 = mlp512(Rf, 'ff2_wT', 'ff2_b', "X2", extra_add=X1)
        ln_bcast(X2, S, 'ln2_g', 'ln2_b')

        sel = sb('sel', trunk)
        pagg = psv.tile([6, D], F32, tag="psv", name="pagg")
        nc.tensor.matmul(pagg, lhsT=sel, rhs=X2, start=True, stop=True)
        agg_d = dram.tile([1, 6 * D], F32)
        agg_sb = ttmp.tile([6, D], F32, tag="aggsb", name="aggsb")
        nc.vector.tensor_copy(out=agg_sb, in_=pagg)
        nc.sync.dma_start(out=agg_d[0:1, :].rearrange("o (r d) -> (o r) d", r=6),
                          in_=agg_sb)

        otherT = sb('otherT', trunk)
        ob1 = sb('other_b1', trunk)
        ow1 = wtrunk.tile([120, 2 * D], F32, tag="ow1", name="ow1", bufs=1)
        wdma(ow1, ins['other_w1T'])
        O1 = trunk.tile([1, 2 * D], F32)
        for half in range(2):
            pv = psv.tile([1, D], F32, tag="psv", name="po1")
            nc.tensor.matmul(pv, lhsT=otherT, rhs=ow1[:, half * D:(half + 1) * D],
                             start=True, stop=False)
            nc.tensor.matmul(pv, lhsT=ones[0:1, 0:1],
                             rhs=ob1[:, half * D:(half + 1) * D],
                             start=False, stop=True)
            nc.scalar.activation(out=O1[:, half * D:(half + 1) * D], in_=pv,
                                 func=AF.Relu)
        o1_d = dram.tile([1, 2 * D], F32)
        nc.sync.dma_start(out=o1_d, in_=O1)
        o1n = ttmp.tile([8, 128], F32, tag="o1n", name="o1n")
        nc.sync.dma_start(out=o1n,
                          in_=o1_d[0:1, :].rearrange("o (r p) -> (o r) p", p=128))
        O1T = trunk.tile([128, 8], F32)
        nc.vector.tensor_copy(out=O1T, in_=trans(o1n, 8, 128))
        ob2 = sb('other_b2', trunk)
        O2 = trunk.tile([1, 2 * D], F32)
        for half in range(2):
            pv = psv.tile([1, D], F32, tag="psv", name="po2")
            for kb in range(2):
                wt = wtrunk.tile([128, 4, D], F32, tag="tw2", name="ow2")
                wdma(wt, ins['other_w2T'][kb * 512:(kb + 1) * 512,
                                          half * D:(half + 1) * D].rearrange(
                    "(a p) d -> p a d", p=128))
                for a in range(4):
                    nc.tensor.matmul(pv, lhsT=O1T[:, kb * 4 + a:kb * 4 + a + 1],
                                     rhs=wt[:, a, :],
                                     start=(kb == 0 and a == 0), stop=False)
            nc.tensor.matmul(pv, lhsT=ones[0:1, 0:1],
                             rhs=ob2[:, half * D:(half + 1) * D],
                             start=False, stop=True)
            nc.scalar.activation(out=O2[:, half * D:(half + 1) * D], in_=pv,
                                 func=AF.Relu)

        XG = trunk.tile([1, D_G], F32, tag="bigx", name="XG", bufs=1)
        nc.sync.dma_start(out=XG[0:1, 0:6 * D], in_=agg_d)
        nc.vector.tensor_copy(out=XG[0:1, 6 * D:8 * D], in_=O2)
        ln_normalize(XG, 1, D_G)
        ln_affine_chunked(XG, 1, D_G, 'lng_g', 'lng_b', ttmp)
        nc.sync.dma_start(out=xg_d, in_=XG)

        ACIN = trunk.tile([S_HAND, D_AC], F32, tag="bigx", name="ACIN", bufs=1)
        xgap = xg_d[:]
        nc.gpsimd.dma_start(out=ACIN[:, 0:D_G],
                            in_=bass.AP(tensor=xgap.tensor, offset=xgap.offset,
                                        ap=[[0, S_HAND]] + xgap.ap[-1:]))
        nc.vector.tensor_copy(out=ACIN[:, D_G:D_AC], in_=X2[0:S_HAND, :])
        ln_normalize(ACIN, S_HAND, D_AC)

        xgn = ttmp.tile([32, 128], F32, tag="xgn", name="xgn")
        nc.sync.dma_start(out=xgn,
                          in_=xg_d[0:1, :].rearrange("o (r p) -> (o r) p", p=128))
        nc.vector.tensor_copy(out=xgT, in_=trans(xgn, 32, 128))
        for k in range(36):
            pt = trans(ACIN[:, k * 128:(k + 1) * 128], S_HAND, 128)
            nc.vector.tensor_copy(out=acT[:, k * S_HAND:(k + 1) * S_HAND], in_=pt)
    # ================= end trunk scope =================

    bounce1 = dram.tile([1, BN], F32)
    gout1 = dram.tile([n_cores, BN], F32)

    def head_ac_layer(lhsT_tiles, w_name, b_name, bounce, pool, blocks):
        """ac head layer. blocks: list of (row0, [(off,sz)...]) DMA batches;
        lhsT_tiles(i, sz) gives the K-tile lhsT AP in running order."""
        b_sb = sb(b_name, tmp, tag="brow", bufs=2)
        pa = ps.tile([S_HAND, 512], F32, tag="ps", name=w_name + "_pa")
        pb = psv.tile([S_HAND, 64], F32, tag="psv", name=w_name + "_pb")
        i = 0
        first = True
        for row0, subs in blocks:
            nfull = sum(1 for (_, sz) in subs if sz == 128)
            wt = pool.tile([128, len(subs), SH_AC], F32, tag="w", name="wt")
            if nfull:
                wdma(wt[:, 0:nfull, :],
                     ins[w_name][row0:row0 + nfull * 128, :].rearrange(
                         "(a p) d -> p a d", p=128))
            if nfull < len(subs):
                off, sz = subs[nfull]
                wdma(wt[0:sz, nfull, :], ins[w_name][row0 + off:row0 + off + sz, :])
            for a, (off, sz) in enumerate(subs):
                lt = lhsT_tiles(i, sz)
                nc.tensor.matmul(pa, lhsT=lt, rhs=wt[0:sz, a, 0:512],
                                 start=first, stop=False)
                nc.tensor.matmul(pb, lhsT=lt, rhs=wt[0:sz, a, 512:SH_AC],
                                 start=first, stop=False)
                first = False
                i += 1
        nc.tensor.matmul(pa, lhsT=ones[0:1, 0:S_HAND], rhs=b_sb[:, 0:512],
                         start=False, stop=True)
        nc.tensor.matmul(pb, lhsT=ones[0:1, 0:S_HAND], rhs=b_sb[:, 512:SH_AC],
                         start=False, stop=True)
        h = tmp.tile([S_HAND, SH_AC], F32, tag="hy", name="h_ac")
        nc.scalar.activation(out=h[:, 0:512], in_=pa, func=AF.Relu)
        nc.scalar.activation(out=h[:, 512:SH_AC], in_=pb, func=AF.Relu)
        if bounce is not None:
            nc.sync.dma_start(
                out=bounce[0:1, OFF_AC:OFF_AC + S_HAND * SH_AC].rearrange(
                    "o (t d) -> (o t) d", t=S_HAND),
                in_=h)
        return h

    def head_g_layer(lhsT_tiles, w_name, b_name, bounce, off, pool):
        b_sb = sb(b_name, tmp, tag="brow", bufs=2)
        pv = psv.tile([1, SH_G], F32, tag="psv", name=w_name + "_pv")
        for kb in range(8):
            wt = pool.tile([128, 4, SH_G], F32, tag="w", name="wt")
            wdma(wt, ins[w_name][kb * 512:(kb + 1) * 512, :].rearrange(
                "(a p) d -> p a d", p=128))
            for a in range(4):
                nc.tensor.matmul(pv, lhsT=lhsT_tiles(kb * 4 + a), rhs=wt[:, a, :],
                                 start=(kb == 0 and a == 0), stop=False)
        nc.tensor.matmul(pv, lhsT=ones[0:1, 0:1], rhs=b_sb, start=False, stop=True)
        h = tmp.tile([1, SH_G], F32, tag="h_g", name="h_g")
        nc.scalar.activation(out=h, in_=pv, func=AF.Relu)
        if bounce is not None:
            nc.sync.dma_start(out=bounce[0:1, off:off + SH_G], in_=h)
        return h

    # W1: uniform 36 K-tiles in 9 batches of 4
    w1_blocks = [(b * 512, [(a * 128, 128) for a in range(4)]) for b in range(9)]
    head_ac_layer(lambda i, sz: acT[:, i * S_HAND:(i + 1) * S_HAND], 'ac_w1T_s',
                  'ac_b1_s', bounce1, wac, w1_blocks)
    head_g_layer(lambda k: xgT[:, k:k + 1], 'am_w1T_s', 'am_b1_s', bounce1,
                 OFF_AM, wg)
    head_g_layer(lambda k: xgT[:, k:k + 1], 'ae_w1T_s', 'ae_b1_s', bounce1,
                 OFF_AE, wg)
    head_g_layer(lambda k: xgT[:, k:k + 1], 'cr_w1T_s', 'cr_b1_s', bounce1,
                 OFF_CR, wg)

    nc.gpsimd.collective_compute(
        "AllGather", ALU.bypass, replica_groups=RG,
        ins=[bounce1[:].opt()], outs=[gout1[:].opt()])

    # load + transpose gathered h1 into lhsT K-tiles
    h1acK = keep.tile([128, n_cores * len(AC_BLK) * S_HAND], F32)
    for c in range(n_cores):
        yc = tmp.tile([S_HAND, SH_AC], F32, tag="hy", name="yc")
        nc.sync.dma_start(
            out=yc,
            in_=gout1[c:c + 1, OFF_AC:OFF_AC + S_HAND * SH_AC].rearrange(
                "o (t d) -> (o t) d", t=S_HAND))
        for j, (off, sz) in enumerate(AC_BLK):
            i = c * len(AC_BLK) + j
            pt = trans(yc[:, off:off + sz], S_HAND, sz)
            nc.vector.tensor_copy(out=h1acK[0:sz, i * S_HAND:(i + 1) * S_HAND],
                                  in_=pt)
    h1gK = {}
    for nm, off in (('am', OFF_AM), ('ae', OFF_AE), ('cr', OFF_CR)):
        gn = tmp.tile([32, 128], F32, tag="gn", name=nm + "gn")
        for c in range(n_cores):
            nc.sync.dma_start(
                out=gn[c * 4:(c + 1) * 4, :],
                in_=gout1[c:c + 1, off:off + SH_G].rearrange(
                    "o (r p) -> (o r) p", p=128))
        t = keep.tile([128, 32], F32, tag="g1" + nm, name="g1" + nm)
        nc.vector.tensor_copy(out=t, in_=trans(gn, 32, 128))
        h1gK[nm] = t

    w1scope.close()  # release W1 weight pools; W2 pools reuse the space
    wac2 = ctx.enter_context(tc.tile_pool(name="wac2", bufs=9))
    wg2 = ctx.enter_context(tc.tile_pool(name="wg2", bufs=9))

    # W2 (K-tiles follow the 8 gathered blocks: 4x128+64 per block)
    w2_blocks = [(c * SH_AC, AC_BLK) for c in range(n_cores)]
    h2ac = head_ac_layer(
        lambda i, sz: h1acK[0:sz, i * S_HAND:(i + 1) * S_HAND], 'ac_w2T_s',
        'ac_b2_s', None, wac2, w2_blocks)
    h2g = {}
    for nm in ('am', 'ae', 'cr'):
        h2g[nm] = head_g_layer(lambda k, _n=nm: h1gK[_n][:, k:k + 1],
                               nm + '_w2T_s', nm + '_b2_s', None, 0, wg2)

    # ---- W3 partials on local h2 slices + tiny AllReduce ----
    arb = dram.tile([1, 24], F32)
    aro = dram.tile([1, 24], F32)  # AR output

    # ac: lpT_partial [2,10] = w3_sT.T @ h2ac^T over the local 576 dims
    w3s = keep.tile([128, 5, 2], F32)
    nc.sync.dma_start(out=w3s[:, 0:4, :],
                      in_=ins['ac_w3T_s'][0:512, :].rearrange(
                          "(j p) o -> p j o", p=128))
    nc.sync.dma_start(out=w3s[0:64, 4, :], in_=ins['ac_w3T_s'][512:576, :])
    plp = psv.tile([2, S_HAND], F32, tag="psv", name="plp")
    for j, (off, sz) in enumerate(AC_BLK):
        pt = trans(h2ac[:, off:off + sz], S_HAND, sz)
        h2T = tmp.tile([128, S_HAND], F32, tag="h2T", name="h2T")
        nc.vector.tensor_copy(out=h2T[0:sz, :], in_=pt)
        nc.tensor.matmul(plp, lhsT=w3s[0:sz, j, :], rhs=h2T[0:sz, :],
                         start=(j == 0), stop=(j == len(AC_BLK) - 1))
    lpT_sb = tmp.tile([2, S_HAND], F32, tag="tail", name="lpT")
    nc.vector.tensor_copy(out=lpT_sb, in_=plp)
    nc.sync.dma_start(out=arb[0:1, 0:20].rearrange("o (a b) -> (o a) b", a=2),
                      in_=lpT_sb)

    # am/ae/cr + pad: elementwise dot with the w3 row shard
    sc4 = tmp.tile([1, 4], F32, tag="sc4", name="sc4", bufs=1)
    nc.vector.memset(sc4, 0.0)
    for si, nm in enumerate(('am', 'ae', 'cr')):
        w3r = sb(nm + '_w3_s', tmp, tag="brow", bufs=2)
        prod = tmp.tile([1, SH_G], F32, tag="tail", name=nm + "prod")
        nc.vector.tensor_mul(out=prod, in0=h2g[nm], in1=w3r)
        nc.vector.reduce_sum(out=sc4[0:1, si:si + 1], in_=prod, axis=AXX.X)
    nc.sync.dma_start(out=arb[0:1, 20:24], in_=sc4)

    nc.gpsimd.collective_compute(
        "AllReduce", ALU.add, replica_groups=RG,
        ins=[arb[:].opt()], outs=[aro[0:1, :].opt()])

    # ---- tail: logits assembly + masked softmax (replicated) ----
    la = keep.tile([1, 24], F32)
    nc.sync.dma_start(out=la, in_=aro[0:1, :])
    logits = keep.tile([1, 22], F32)
    nc.vector.tensor_copy(out=logits, in_=la[0:1, 0:22])
    mb = sb('maskbias22', keep)
    nc.vector.tensor_add(out=logits, in0=logits, in1=mb)
    mx = keep.tile([1, 1], F32, name="fmx")
    nc.vector.reduce_max(out=mx, in_=logits, axis=AXX.X)
    nc.vector.tensor_scalar_sub(out=logits, in0=logits, scalar1=mx)
    nc.scalar.activation(out=logits, in_=logits, func=AF.Exp)
    sm = keep.tile([1, 1], F32, name="fsm")
    nc.vector.reduce_sum(out=sm, in_=logits, axis=AXX.X)
    nc.vector.reciprocal(out=sm, in_=sm)
    nc.vector.tensor_scalar_mul(out=logits, in0=logits, scalar1=sm)
    nc.sync.dma_start(out=probs_out, in_=logits)
    crb3 = sb('cr_b3', keep)
    val = keep.tile([1, 1], F32, name="val")
    nc.vector.tensor_add(out=val, in0=la[0:1, 22:23], in1=crb3)
    nc.sync.dma_start(out=value_out, in_=val)


def build_program(n_cores=NCORES):
    nc = bacc.Bacc("TRN2", target_bir_lowering=False, debug=False,
                   num_devices=n_cores)
    ins = {}
    for name, shape in INPUT_SPEC.items():
        ins[name] = nc.dram_tensor(name, list(shape), F32, kind="ExternalInput").ap()
    probs = nc.dram_tensor("probs", [1, 22], F32, kind="ExternalOutput").ap()
    value = nc.dram_tensor("value", [1, 1], F32, kind="ExternalOutput").ap()
    with tile.TileContext(nc) as tc:
        with ExitStack() as ctx:
            emit(ctx, tc, ins, probs, value, n_cores=n_cores)
    nc.compile()
    return nc


_PROG = {}


def _get_program(n_cores=NCORES):
    if n_cores not in _PROG:
        _PROG[n_cores] = build_program(n_cores)
    return _PROG[n_cores]


def kernel(**inputs):
    per_core = host_prep(**inputs)
    nc = _get_program(NCORES)
    res = run_bass_kernel_spmd(nc, per_core, core_ids=list(range(NCORES)))
    out = res.results[0]
    probs = np.asarray(out['probs'], np.float32).reshape(22)
    value = np.asarray(out['value'], np.float32).reshape(1)
    return probs, value


# revision 13
# speedup vs baseline: 1.2347x; 1.1483x over previous
"""Trainium2 Bass kernel for nn_ActorCritic forward (8-core tensor-parallel).

Strategy (memory-bound problem — ~573MB of head weights dominate):
  - Trunk (embedding + one shared transformer block over the 55 concatenated
    hand/draw/disc tokens with block-diagonal attention masking, other-MLP,
    global feature assembly + layernorms) is small and replicated on all
    8 cores.
  - Head MLPs (ac: 4608^2 x2, am/ae/cr: 4096^2 x2) are column-parallel
    across the 8 cores: each core computes a 1/8 slice of
    h1 = relu(W1 x + b1), one AllGather rebuilds h1 everywhere, each core
    computes its h2 slice, then each core contracts its h2 slice with its
    W3 column-shard and a tiny AllReduce (24 floats) sums the partial
    logits/value. The masked softmax tail runs redundantly on every core.
  - Weights ride the PE's MOVING operand (rhs) so weight bytes stream
    HBM->SBUF->PE at DMA rate; weight DMAs are batched 4 K-tiles per
    dma_start (~1MB each) and alternate between the two HWDGE rings
    (sync + scalar engines) to amortize issue cost.
  - The lnac affine is folded into ac W1 on the host; head-layer biases are
    rank-1 matmul accumulates; W3 biases are folded into the tail mask add.
"""

import numpy as np
from contextlib import ExitStack

import concourse.bass as bass
import concourse.bacc as bacc
import concourse.tile as tile
from concourse import mybir
from concourse.bass_utils import run_bass_kernel_spmd
from concourse.masks import make_identity

F32 = mybir.dt.float32
AF = mybir.ActivationFunctionType
ALU = mybir.AluOpType
AXX = mybir.AxisListType
F32R = mybir.dt.float32r


def _r(ap):
    """Tiles feeding the big matmuls are allocated as float32r natively."""
    return ap

NCORES = 8
D = 512
S_HAND, S_DRAW, S_DISC = 10, 30, 15
S = S_HAND + S_DRAW + S_DISC          # 55
NH, HD = 8, 64
EPS = 1e-5
D_AC = 9 * D                           # 4608
D_G = 8 * D                            # 4096
SH_AC = D_AC // NCORES                 # 576
SH_G = D_G // NCORES                   # 512
NEG = -1e30
BN = 7296                              # AG1 bounce: 10*576 + 3*512
OFF_AC, OFF_AM, OFF_AE, OFF_CR = 0, 5760, 6272, 6784
AC_BLK = [(j * 128, min(128, SH_AC - j * 128)) for j in range((SH_AC + 127) // 128)]

# inputs that feed the PE as float32r (full-rate fp32 matmul mode)
R32_INPUTS = {
    'cardsT', 'otherT', 'emb_wT', 'emb_b', 'inproj_wT', 'inproj_b',
    'outproj_wT', 'outproj_b', 'ff1_wT', 'ff1_b', 'ff2_wT', 'ff2_b',
    'other_w1T', 'other_b1', 'other_w2T', 'other_b2',
    'ac_w1T_s', 'ac_b1_s', 'ac_w2T_s', 'ac_b2_s',
    'am_w1T_s', 'am_b1_s', 'am_w2T_s', 'am_b2_s',
    'ae_w1T_s', 'ae_b1_s', 'ae_w2T_s', 'ae_b2_s',
    'cr_w1T_s', 'cr_b1_s', 'cr_w2T_s', 'cr_b2_s',
}

INPUT_SPEC = {
    'cardsT': (128, S), 'sel': (S, 6), 'maskfull': (S, S), 'maskbias22': (1, 22),
    'otherT': (120, 1), 'cr_b3': (1, 1),
    'emb_wT': (128, D), 'emb_b': (1, D),
    'inproj_wT': (D, 3 * D), 'inproj_b': (1, 3 * D),
    'outproj_wT': (D, D), 'outproj_b': (1, D),
    'ln1_g': (D,), 'ln1_b': (D,),
    'ff1_wT': (D, D), 'ff1_b': (1, D),
    'ff2_wT': (D, D), 'ff2_b': (1, D),
    'ln2_g': (D,), 'ln2_b': (D,),
    'other_w1T': (120, 2 * D), 'other_b1': (1, 2 * D),
    'other_w2T': (2 * D, 2 * D), 'other_b2': (1, 2 * D),
    'lng_g': (D_G,), 'lng_b': (D_G,),
    'ac_w1T_s': (D_AC, SH_AC), 'ac_b1_s': (1, SH_AC),
    'ac_w2T_s': (D_AC, SH_AC), 'ac_b2_s': (1, SH_AC),
    'ac_w3T_s': (SH_AC, 2),
    'am_w1T_s': (D_G, SH_G), 'am_b1_s': (1, SH_G),
    'am_w2T_s': (D_G, SH_G), 'am_b2_s': (1, SH_G),
    'am_w3_s': (1, SH_G),
    'ae_w1T_s': (D_G, SH_G), 'ae_b1_s': (1, SH_G),
    'ae_w2T_s': (D_G, SH_G), 'ae_b2_s': (1, SH_G),
    'ae_w3_s': (1, SH_G),
    'cr_w1T_s': (D_G, SH_G), 'cr_b1_s': (1, SH_G),
    'cr_w2T_s': (D_G, SH_G), 'cr_b2_s': (1, SH_G),
    'cr_w3_s': (1, SH_G),
}


def host_prep(hand, draw_pile, disc_pile, character, monster, energy, params,
              hand_size, valid_action_mask):
    """Build the 8 per-core device input dicts from the full inputs."""
    p = {k: np.asarray(v, np.float32) for k, v in params.items()}
    f32 = np.float32

    cards = np.concatenate([np.asarray(hand, f32), np.asarray(draw_pile, f32),
                            np.asarray(disc_pile, f32)], 0)
    cardsT = np.ascontiguousarray(cards.T)

    maskfull = np.full((S, S), NEG, f32)
    maskfull[:S_HAND, :int(hand_size)] = 0.0
    maskfull[S_HAND:S_HAND + S_DRAW, S_HAND:S_HAND + S_DRAW] = 0.0
    maskfull[S_HAND + S_DRAW:, S_HAND + S_DRAW:] = 0.0

    sel = np.zeros((S, 6), f32)
    sel[:S_HAND, 0] = 1.0 / S_HAND
    sel[:S_HAND, 1] = 1.0
    sel[S_HAND:S_HAND + S_DRAW, 2] = 1.0 / S_DRAW
    sel[S_HAND:S_HAND + S_DRAW, 3] = 1.0
    sel[S_HAND + S_DRAW:, 4] = 1.0 / S_DISC
    sel[S_HAND + S_DRAW:, 5] = 1.0

    # W3 biases are folded into the mask-add before the softmax
    maskbias22 = np.where(np.asarray(valid_action_mask) == 0, NEG, 0.0).astype(f32)
    maskbias22[0:10] += p['ac_b3'][0]
    maskbias22[10:20] += p['ac_b3'][1]
    maskbias22[20] += p['am_b3'][0]
    maskbias22[21] += p['ae_b3'][0]

    # fold the lnac affine into the ac head's first layer:
    #   W1 @ (g*xn + b) + b1 == (W1*g) @ xn + (b1 + W1 @ b)
    ac_w1g = p['ac_w1'] * p['lnac_g'][None, :]
    ac_b1f = p['ac_b1'] + p['ac_w1'] @ p['lnac_b']

    shared = dict(
        cardsT=cardsT, sel=sel, maskfull=maskfull, maskbias22=maskbias22[None, :],
        otherT=np.concatenate([np.asarray(character, f32), np.asarray(monster, f32),
                               np.asarray(energy, f32)])[:, None],
        cr_b3=p['cr_b3'][None, :],
        emb_wT=np.ascontiguousarray(p['emb_card_w'].T), emb_b=p['emb_card_b'][None, :],
        inproj_wT=np.ascontiguousarray(p['in_proj_w'].T), inproj_b=p['in_proj_b'][None, :],
        outproj_wT=np.ascontiguousarray(p['out_proj_w'].T), outproj_b=p['out_proj_b'][None, :],
        ln1_g=p['ln1_g'], ln1_b=p['ln1_b'],
        ff1_wT=np.ascontiguousarray(p['ff1_w'].T), ff1_b=p['ff1_b'][None, :],
        ff2_wT=np.ascontiguousarray(p['ff2_w'].T), ff2_b=p['ff2_b'][None, :],
        ln2_g=p['ln2_g'], ln2_b=p['ln2_b'],
        other_w1T=np.ascontiguousarray(p['other_w1'].T), other_b1=p['other_b1'][None, :],
        other_w2T=np.ascontiguousarray(p['other_w2'].T), other_b2=p['other_b2'][None, :],
        lng_g=p['lng_g'], lng_b=p['lng_b'],
    )
    w1 = {'ac': ac_w1g, 'am': p['am_w1'], 'ae': p['ae_w1'], 'cr': p['cr_w1']}
    b1 = {'ac': ac_b1f, 'am': p['am_b1'], 'ae': p['ae_b1'], 'cr': p['cr_b1']}

    per_core = []
    for c in range(NCORES):
        m = dict(shared)
        for n, sh in (('ac', SH_AC), ('am', SH_G), ('ae', SH_G), ('cr', SH_G)):
            sl = slice(c * sh, (c + 1) * sh)
            m[n + '_w1T_s'] = np.ascontiguousarray(w1[n].T[:, sl])
            m[n + '_b1_s'] = b1[n][None, sl]
            m[n + '_w2T_s'] = np.ascontiguousarray(p[n + '_w2'].T[:, sl])
            m[n + '_b2_s'] = p[n + '_b2'][None, sl]
            if n == 'ac':
                m['ac_w3T_s'] = np.ascontiguousarray(p['ac_w3'].T[sl, :])
            else:
                m[n + '_w3_s'] = p[n + '_w3'][:, sl].reshape(1, sh)
        per_core.append(m)
    return per_core


def emit(ctx: ExitStack, tc: tile.TileContext, ins, probs_out, value_out,
         n_cores=NCORES):
    nc = tc.nc
    RG = [list(range(n_cores))]

    consts = ctx.enter_context(tc.tile_pool(name="consts", bufs=1))
    keep = ctx.enter_context(tc.tile_pool(name="keep", bufs=1))
    tmp = ctx.enter_context(tc.tile_pool(name="tmp", bufs=2))
    ps = ctx.enter_context(tc.tile_pool(name="ps", bufs=3, space="PSUM"))
    pst = ctx.enter_context(tc.tile_pool(name="pst", bufs=2, space="PSUM"))
    psv = ctx.enter_context(tc.tile_pool(name="psv", bufs=2, space="PSUM"))
    dram = ctx.enter_context(tc.tile_pool(name="dram", bufs=1, space="DRAM"))

    ident = consts.tile([128, 128], F32)
    make_identity(nc, ident)
    ones_f = consts.tile([1, 64], F32)
    nc.vector.memset(ones_f, 1.0)
    ones = consts.tile([1, 64], F32R)
    nc.vector.tensor_copy(out=ones, in_=ones_f)
    eps55 = consts.tile([S, 1], F32)
    nc.vector.memset(eps55, EPS)

    # alternate big weight DMAs across the two HWDGE rings
    _dma_eng = [0]

    def wdma(out, in_):
        eng = nc.sync if _dma_eng[0] % 2 == 0 else nc.scalar
        _dma_eng[0] += 1
        eng.dma_start(out=out, in_=in_)

    def sb(name, pool, tag=None, bufs=None):
        shp = list(INPUT_SPEC[name])
        if len(shp) == 1:
            shp = [1] + shp
        dt = F32R if name in R32_INPUTS else F32
        t = pool.tile(shp, dt, tag=tag or name, name=name + "_sb", bufs=bufs)
        src = ins[name]
        if len(INPUT_SPEC[name]) == 1:
            src = src.rearrange("(o n) -> o n", o=1)
        nc.sync.dma_start(out=t, in_=src)
        return t

    def trans(in_ap, P, Fr):
        pt = pst.tile([Fr, P], F32, tag="ptr", name="ptr")
        nc.tensor.transpose(pt, in_ap, ident[0:P, 0:P])
        return pt

    def ln_normalize(x, P, Dm):
        nsub = Dm // 512
        stats = tmp.tile([P, nsub, 6], F32, tag="lnstats", name="lnstats")
        xs = x.rearrange("p (n d) -> p n d", n=nsub) if nsub > 1 else x
        for i in range(nsub):
            nc.vector.bn_stats(out=stats[:, i, :],
                               in_=(xs[:, i, :] if nsub > 1 else xs))
        mv = tmp.tile([P, 2], F32, tag="lnmv", name="lnmv")
        nc.vector.bn_aggr(out=mv, in_=stats)
        nc.scalar.activation(out=mv[:, 1:2], in_=mv[:, 1:2], func=AF.Sqrt,
                             bias=eps55[0:P], scale=1.0)
        nc.vector.reciprocal(out=mv[:, 1:2], in_=mv[:, 1:2])
        nc.vector.tensor_scalar(out=x, in0=x, scalar1=mv[:, 0:1], scalar2=mv[:, 1:2],
                                op0=ALU.subtract, op1=ALU.mult)

    def ln_affine_chunked(x, P, Dm, g_name, b_name, pool):
        for j in range(Dm // 512):
            gt = pool.tile([P, 512], F32, tag="lnaff", name="lnaff_g", bufs=3)
            bt = pool.tile([P, 512], F32, tag="lnaff", name="lnaff_b", bufs=3)
            for t, nm in ((gt, g_name), (bt, b_name)):
                src = ins[nm][j * 512:(j + 1) * 512]
                ap = bass.AP(tensor=src.tensor, offset=src.offset,
                             ap=[[0, P]] + src.ap)
                nc.gpsimd.dma_start(out=t, in_=ap)
            sl = x[:, j * 512:(j + 1) * 512]
            nc.vector.tensor_mul(out=sl, in0=sl, in1=gt)
            nc.vector.tensor_add(out=sl, in0=sl, in1=bt)

    acT = keep.tile([128, 36 * S_HAND], F32R)       # ac_in^T K-tiles
    xgT = keep.tile([128, 32], F32R)                # x_global^T K-tiles
    xg_d = dram.tile([1, D_G], F32)

    # ================= trunk + W1 (scoped pools) =================
    w1scope = ExitStack()
    wac = w1scope.enter_context(tc.tile_pool(name="wac", bufs=4))
    wg = w1scope.enter_context(tc.tile_pool(name="wg", bufs=3))
    with tc.tile_pool(name="trunk", bufs=1) as trunk, \
         tc.tile_pool(name="ttmp", bufs=2) as ttmp, \
         tc.tile_pool(name="wtrunk", bufs=2) as wtrunk:

        def act55(name):
            return trunk.tile([S, D], F32, tag="act55", name=name, bufs=3)

        cardsT = sb('cardsT', trunk)
        emb_wT = sb('emb_wT', trunk)
        emb_b = sb('emb_b', trunk, tag="brow", bufs=3)
        px = ps.tile([S, D], F32, tag="ps", name="px")
        nc.tensor.matmul(px, lhsT=_r(cardsT), rhs=_r(emb_wT), start=True,
                         stop=False)
        nc.tensor.matmul(px, lhsT=_r(ones[0:1, 0:S]), rhs=_r(emb_b), start=False,
                         stop=True)
        X0 = act55("X0")
        nc.vector.tensor_copy(out=X0, in_=px)

        X0T = trunk.tile([128, 4 * S], F32R)
        for k in range(4):
            pt = trans(X0[:, k * 128:(k + 1) * 128], S, 128)
            nc.vector.tensor_copy(out=X0T[:, k * S:(k + 1) * S], in_=pt)

        inproj_b = sb('inproj_b', trunk, tag='ipb', bufs=1)
        pqkv = [ps.tile([S, D], F32, tag="ps", name=f"pqkv{qi}") for qi in range(3)]
        for k in range(4):
            wt = wtrunk.tile([128, 3 * D], F32R, tag="tw", name="tw")
            wdma(wt, ins['inproj_wT'][k * 128:(k + 1) * 128, :])
            for qi in range(3):
                nc.tensor.matmul(pqkv[qi], lhsT=_r(X0T[:, k * S:(k + 1) * S]),
                                 rhs=_r(wt[:, qi * D:(qi + 1) * D]),
                                 start=(k == 0), stop=False)
        QKV = []
        for qi in range(3):
            nc.tensor.matmul(pqkv[qi], lhsT=_r(ones[0:1, 0:S]),
                             rhs=_r(inproj_b[:, qi * D:(qi + 1) * D]),
                             start=False, stop=True)
            t = trunk.tile([S, D], F32, tag=f"qkv{qi}", name=f"qkv{qi}")
            nc.vector.tensor_copy(out=t, in_=pqkv[qi])
            QKV.append(t)
        Q, K, V = QKV

        # attention: full-Q/K transposes, per-head scores, batched softmax
        qTf = trunk.tile([128, 4 * S], F32)
        kTf = trunk.tile([128, 4 * S], F32)
        for k in range(4):
            nc.vector.tensor_copy(out=qTf[:, k * S:(k + 1) * S],
                                  in_=trans(Q[:, k * 128:(k + 1) * 128], S, 128))
            nc.vector.tensor_copy(out=kTf[:, k * S:(k + 1) * S],
                                  in_=trans(K[:, k * 128:(k + 1) * 128], S, 128))

        maskfull = sb('maskfull', trunk)
        SALL = trunk.tile([S, NH, S], F32)
        for h in range(NH):
            bp = (h % 2) * HD
            blk = slice((h // 2) * S, (h // 2 + 1) * S)
            psc = pst.tile([S, S], F32, tag="ptr", name="psc")
            nc.tensor.matmul(psc, lhsT=qTf[bp:bp + HD, blk],
                             rhs=kTf[bp:bp + HD, blk], start=True, stop=True)
            nc.scalar.activation(out=SALL[:, h, :], in_=psc, func=AF.Copy,
                                 scale=0.125)
        mfap = maskfull[:]
        nc.vector.tensor_add(
            out=SALL, in0=SALL,
            in1=bass.AP(tensor=mfap.tensor, offset=mfap.offset,
                        ap=[mfap.ap[0], [0, NH], mfap.ap[1]]))
        mx8 = ttmp.tile([S, NH], F32, tag="mx8", name="mx8")
        nc.vector.reduce_max(out=mx8, in_=SALL, axis=AXX.X)
        mxap = mx8[:]
        nc.vector.tensor_tensor(
            out=SALL, in0=SALL,
            in1=bass.AP(tensor=mxap.tensor, offset=mxap.offset,
                        ap=[mxap.ap[0], mxap.ap[1], [0, S]]),
            op=ALU.subtract)
        nc.scalar.activation(out=SALL, in_=SALL, func=AF.Exp)
        sm8 = ttmp.tile([S, NH], F32, tag="sm8", name="sm8")
        nc.vector.reduce_sum(out=sm8, in_=SALL, axis=AXX.X)
        nc.vector.reciprocal(out=sm8, in_=sm8)
        smap = sm8[:]
        nc.vector.tensor_tensor(
            out=SALL, in0=SALL,
            in1=bass.AP(tensor=smap.tensor, offset=smap.offset,
                        ap=[smap.ap[0], smap.ap[1], [0, S]]),
            op=ALU.mult)
        AOp = ps.tile([S, D], F32, tag="ao", name="AOp", bufs=1)
        for h in range(NH):
            hs = slice(h * HD, (h + 1) * HD)
            aT = ttmp.tile([S, S], F32, tag="aT", name="aT")
            nc.vector.tensor_copy(out=aT, in_=trans(SALL[:, h, :], S, S))
            nc.tensor.matmul(AOp[:, hs], lhsT=aT, rhs=V[:, hs], start=True,
                             stop=True)
        AO = act55("AO")
        nc.vector.tensor_copy(out=AO, in_=AOp)

        def ln_bcast(x, P, g_name, b_name):
            ln_normalize(x, P, D)
            ln_affine_chunked(x, P, D, g_name, b_name, ttmp)

        def mlp512(x_sb, wT_name, b_name, out_name, extra_add=None):
            xT = trunk.tile([128, 4 * S], F32R, tag="xT", name=wT_name + "_xT",
                            bufs=2)
            for k in range(4):
                pt = trans(x_sb[:, k * 128:(k + 1) * 128], S, 128)
                nc.vector.tensor_copy(out=xT[:, k * S:(k + 1) * S], in_=pt)
            b_sb = sb(b_name, trunk, tag="brow", bufs=3)
            po = ps.tile([S, D], F32, tag="ps", name=wT_name + "_po")
            wt = wtrunk.tile([128, 4, D], F32R, tag="tw2", name=wT_name + "_w")
            wdma(wt, ins[wT_name][:].rearrange("(a p) d -> p a d", p=128))
            for k in range(4):
                nc.tensor.matmul(po, lhsT=_r(xT[:, k * S:(k + 1) * S]),
                                 rhs=_r(wt[:, k, :]), start=(k == 0), stop=False)
            nc.tensor.matmul(po, lhsT=_r(ones[0:1, 0:S]), rhs=_r(b_sb),
                             start=False, stop=True)
            o = act55(out_name)
            if extra_add is not None:
                nc.vector.tensor_add(out=o, in0=po, in1=extra_add)
            else:
                nc.vector.tensor_copy(out=o, in_=po)
            return o

        X1 = mlp512(AO, 'outproj_wT', 'outproj_b', "X1", extra_add=X0)
        ln_bcast(X1, S, 'ln1_g', 'ln1_b')
        Rf = mlp512(X1, 'ff1_wT', 'ff1_b', "Rf")
        nc.scalar.activation(out=Rf, in_=Rf, func=AF.Relu)
        X2 = mlp512(Rf, 'ff2_wT', 'ff2_b', "X2", extra_add=X1)
        ln_bcast(X2, S, 'ln2_g', 'ln2_b')

        sel = sb('sel', trunk)
        pagg = psv.tile([6, D], F32, tag="psv", name="pagg")
        nc.tensor.matmul(pagg, lhsT=sel, rhs=X2, start=True, stop=True)
        agg_d = dram.tile([1, 6 * D], F32)
        agg_sb = ttmp.tile([6, D], F32, tag="aggsb", name="aggsb")
        nc.vector.tensor_copy(out=agg_sb, in_=pagg)
        nc.sync.dma_start(out=agg_d[0:1, :].rearrange("o (r d) -> (o r) d", r=6),
                          in_=agg_sb)

        otherT = sb('otherT', trunk)
        ob1 = sb('other_b1', trunk)
        ow1 = wtrunk.tile([120, 2 * D], F32R, tag="ow1", name="ow1", bufs=1)
        wdma(ow1, ins['other_w1T'])
        O1 = trunk.tile([1, 2 * D], F32)
        for half in range(2):
            pv = psv.tile([1, D], F32, tag="psv", name="po1")
            nc.tensor.matmul(pv, lhsT=_r(otherT),
                             rhs=_r(ow1[:, half * D:(half + 1) * D]),
                             start=True, stop=False)
            nc.tensor.matmul(pv, lhsT=_r(ones[0:1, 0:1]),
                             rhs=_r(ob1[:, half * D:(half + 1) * D]),
                             start=False, stop=True)
            nc.scalar.activation(out=O1[:, half * D:(half + 1) * D], in_=pv,
                                 func=AF.Relu)
        o1_d = dram.tile([1, 2 * D], F32)
        nc.sync.dma_start(out=o1_d, in_=O1)
        o1n = ttmp.tile([8, 128], F32, tag="o1n", name="o1n")
        nc.sync.dma_start(out=o1n,
                          in_=o1_d[0:1, :].rearrange("o (r p) -> (o r) p", p=128))
        O1T = trunk.tile([128, 8], F32R)
        nc.vector.tensor_copy(out=O1T, in_=trans(o1n, 8, 128))
        ob2 = sb('other_b2', trunk)
        O2 = trunk.tile([1, 2 * D], F32)
        for half in range(2):
            pv = psv.tile([1, D], F32, tag="psv", name="po2")
            for kb in range(2):
                wt = wtrunk.tile([128, 4, D], F32R, tag="tw2", name="ow2")
                wdma(wt, ins['other_w2T'][kb * 512:(kb + 1) * 512,
                                          half * D:(half + 1) * D].rearrange(
                    "(a p) d -> p a d", p=128))
                for a in range(4):
                    nc.tensor.matmul(pv,
                                     lhsT=_r(O1T[:, kb * 4 + a:kb * 4 + a + 1]),
                                     rhs=_r(wt[:, a, :]),
                                     start=(kb == 0 and a == 0), stop=False)
            nc.tensor.matmul(pv, lhsT=_r(ones[0:1, 0:1]),
                             rhs=_r(ob2[:, half * D:(half + 1) * D]),
                             start=False, stop=True)
            nc.scalar.activation(out=O2[:, half * D:(half + 1) * D], in_=pv,
                                 func=AF.Relu)

        XG = trunk.tile([1, D_G], F32, tag="bigx", name="XG", bufs=1)
        nc.sync.dma_start(out=XG[0:1, 0:6 * D], in_=agg_d)
        nc.vector.tensor_copy(out=XG[0:1, 6 * D:8 * D], in_=O2)
        ln_normalize(XG, 1, D_G)
        ln_affine_chunked(XG, 1, D_G, 'lng_g', 'lng_b', ttmp)
        nc.sync.dma_start(out=xg_d, in_=XG)

        ACIN = trunk.tile([S_HAND, D_AC], F32, tag="bigx", name="ACIN", bufs=1)
        xgap = xg_d[:]
        nc.gpsimd.dma_start(out=ACIN[:, 0:D_G],
                            in_=bass.AP(tensor=xgap.tensor, offset=xgap.offset,
                                        ap=[[0, S_HAND]] + xgap.ap[-1:]))
        nc.vector.tensor_copy(out=ACIN[:, D_G:D_AC], in_=X2[0:S_HAND, :])
        ln_normalize(ACIN, S_HAND, D_AC)

        xgn = ttmp.tile([32, 128], F32, tag="xgn", name="xgn")
        nc.sync.dma_start(out=xgn,
                          in_=xg_d[0:1, :].rearrange("o (r p) -> (o r) p", p=128))
        nc.vector.tensor_copy(out=xgT, in_=trans(xgn, 32, 128))
        for k in range(36):
            pt = trans(ACIN[:, k * 128:(k + 1) * 128], S_HAND, 128)
            nc.vector.tensor_copy(out=acT[:, k * S_HAND:(k + 1) * S_HAND], in_=pt)
    # ================= end trunk scope =================

    bounce1 = dram.tile([1, BN], F32)
    gout1 = dram.tile([n_cores, BN], F32)

    def head_ac_layer(lhsT_tiles, w_name, b_name, bounce, pool, blocks):
        """ac head layer. blocks: list of (row0, [(off,sz)...]) DMA batches;
        lhsT_tiles(i, sz) gives the K-tile lhsT AP in running order."""
        b_sb = sb(b_name, tmp, tag="brow", bufs=2)
        HA = SH_AC // 2
        pa = ps.tile([S_HAND, HA], F32, tag="ps", name=w_name + "_pa")
        pb = psv.tile([S_HAND, HA], F32, tag="psv", name=w_name + "_pb")
        i = 0
        first = True
        for row0, subs in blocks:
            nfull = sum(1 for (_, sz) in subs if sz == 128)
            wt = pool.tile([128, len(subs), SH_AC], F32R, tag="w", name="wt")
            if nfull:
                wdma(wt[:, 0:nfull, :],
                     ins[w_name][row0:row0 + nfull * 128, :].rearrange(
                         "(a p) d -> p a d", p=128))
            if nfull < len(subs):
                off, sz = subs[nfull]
                wdma(wt[0:sz, nfull, :], ins[w_name][row0 + off:row0 + off + sz, :])
            for a, (off, sz) in enumerate(subs):
                lt = _r(lhsT_tiles(i, sz))
                nc.tensor.matmul(pa, lhsT=lt, rhs=_r(wt[0:sz, a, 0:HA]),
                                 start=first, stop=False)
                nc.tensor.matmul(pb, lhsT=lt, rhs=_r(wt[0:sz, a, HA:SH_AC]),
                                 start=first, stop=False)
                first = False
                i += 1
        nc.tensor.matmul(pa, lhsT=_r(ones[0:1, 0:S_HAND]), rhs=_r(b_sb[:, 0:HA]),
                         start=False, stop=True)
        nc.tensor.matmul(pb, lhsT=_r(ones[0:1, 0:S_HAND]),
                         rhs=_r(b_sb[:, HA:SH_AC]), start=False, stop=True)
        h = tmp.tile([S_HAND, SH_AC], F32, tag="hy", name="h_ac")
        nc.scalar.activation(out=h[:, 0:HA], in_=pa, func=AF.Relu)
        nc.scalar.activation(out=h[:, HA:SH_AC], in_=pb, func=AF.Relu)
        if bounce is not None:
            nc.sync.dma_start(
                out=bounce[0:1, OFF_AC:OFF_AC + S_HAND * SH_AC].rearrange(
                    "o (t d) -> (o t) d", t=S_HAND),
                in_=h)
        return h

    def head_g_layer(lhsT_tiles, w_name, b_name, bounce, off, pool):
        b_sb = sb(b_name, tmp, tag="brow", bufs=2)
        pv = psv.tile([1, SH_G], F32, tag="psv", name=w_name + "_pv")
        for kb in range(8):
            wt = pool.tile([128, 4, SH_G], F32R, tag="w", name="wt")
            wdma(wt, ins[w_name][kb * 512:(kb + 1) * 512, :].rearrange(
                "(a p) d -> p a d", p=128))
            for a in range(4):
                nc.tensor.matmul(pv, lhsT=_r(lhsT_tiles(kb * 4 + a)),
                                 rhs=_r(wt[:, a, :]),
                                 start=(kb == 0 and a == 0), stop=False)
        nc.tensor.matmul(pv, lhsT=_r(ones[0:1, 0:1]), rhs=_r(b_sb), start=False,
                         stop=True)
        h = tmp.tile([1, SH_G], F32, tag="h_g", name="h_g")
        nc.scalar.activation(out=h, in_=pv, func=AF.Relu)
        if bounce is not None:
            nc.sync.dma_start(out=bounce[0:1, off:off + SH_G], in_=h)
        return h

    # W1: uniform 36 K-tiles in 9 batches of 4
    w1_blocks = [(b * 512, [(a * 128, 128) for a in range(4)]) for b in range(9)]
    head_ac_layer(lambda i, sz: acT[:, i * S_HAND:(i + 1) * S_HAND], 'ac_w1T_s',
                  'ac_b1_s', bounce1, wac, w1_blocks)
    head_g_layer(lambda k: xgT[:, k:k + 1], 'am_w1T_s', 'am_b1_s', bounce1,
                 OFF_AM, wg)
    head_g_layer(lambda k: xgT[:, k:k + 1], 'ae_w1T_s', 'ae_b1_s', bounce1,
                 OFF_AE, wg)
    head_g_layer(lambda k: xgT[:, k:k + 1], 'cr_w1T_s', 'cr_b1_s', bounce1,
                 OFF_CR, wg)

    nc.gpsimd.collective_compute(
        "AllGather", ALU.bypass, replica_groups=RG,
        ins=[bounce1[:].opt()], outs=[gout1[:].opt()])

    # load + transpose gathered h1 into lhsT K-tiles
    h1acK = keep.tile([128, n_cores * len(AC_BLK) * S_HAND], F32R)
    for c in range(n_cores):
        yc = tmp.tile([S_HAND, SH_AC], F32, tag="hy", name="yc")
        nc.sync.dma_start(
            out=yc,
            in_=gout1[c:c + 1, OFF_AC:OFF_AC + S_HAND * SH_AC].rearrange(
                "o (t d) -> (o t) d", t=S_HAND))
        for j, (off, sz) in enumerate(AC_BLK):
            i = c * len(AC_BLK) + j
            pt = trans(yc[:, off:off + sz], S_HAND, sz)
            nc.vector.tensor_copy(out=h1acK[0:sz, i * S_HAND:(i + 1) * S_HAND],
                                  in_=pt)
    h1gK = {}
    for nm, off in (('am', OFF_AM), ('ae', OFF_AE), ('cr', OFF_CR)):
        gn = tmp.tile([32, 128], F32, tag="gn", name=nm + "gn")
        for c in range(n_cores):
            nc.sync.dma_start(
                out=gn[c * 4:(c + 1) * 4, :],
                in_=gout1[c:c + 1, off:off + SH_G].rearrange(
                    "o (r p) -> (o r) p", p=128))
        t = keep.tile([128, 32], F32R, tag="g1" + nm, name="g1" + nm)
        nc.vector.tensor_copy(out=t, in_=trans(gn, 32, 128))
        h1gK[nm] = t

    w1scope.close()  # release W1 weight pools; W2 pools reuse the space
    wac2 = ctx.enter_context(tc.tile_pool(name="wac2", bufs=9))
    wg2 = ctx.enter_context(tc.tile_pool(name="wg2", bufs=9))

    # W2 (K-tiles follow the 8 gathered blocks: 4x128+64 per block)
    w2_blocks = [(c * SH_AC, AC_BLK) for c in range(n_cores)]
    h2ac = head_ac_layer(
        lambda i, sz: h1acK[0:sz, i * S_HAND:(i + 1) * S_HAND], 'ac_w2T_s',
        'ac_b2_s', None, wac2, w2_blocks)
    h2g = {}
    for nm in ('am', 'ae', 'cr'):
        h2g[nm] = head_g_layer(lambda k, _n=nm: h1gK[_n][:, k:k + 1],
                               nm + '_w2T_s', nm + '_b2_s', None, 0, wg2)

    # ---- W3 partials on local h2 slices + tiny AllReduce ----
    arb = dram.tile([1, 24], F32)
    aro = dram.tile([1, 24], F32)  # AR output

    # ac: lpT_partial [2,10] = w3_sT.T @ h2ac^T over the local 576 dims
    w3s = keep.tile([128, 5, 2], F32)
    nc.sync.dma_start(out=w3s[:, 0:4, :],
                      in_=ins['ac_w3T_s'][0:512, :].rearrange(
                          "(j p) o -> p j o", p=128))
    nc.sync.dma_start(out=w3s[0:64, 4, :], in_=ins['ac_w3T_s'][512:576, :])
    plp = psv.tile([2, S_HAND], F32, tag="psv", name="plp")
    for j, (off, sz) in enumerate(AC_BLK):
        pt = trans(h2ac[:, off:off + sz], S_HAND, sz)
        h2T = tmp.tile([128, S_HAND], F32, tag="h2T", name="h2T")
        nc.vector.tensor_copy(out=h2T[0:sz, :], in_=pt)
        nc.tensor.matmul(plp, lhsT=w3s[0:sz, j, :], rhs=h2T[0:sz, :],
                         start=(j == 0), stop=(j == len(AC_BLK) - 1))
    lpT_sb = tmp.tile([2, S_HAND], F32, tag="tail", name="lpT")
    nc.vector.tensor_copy(out=lpT_sb, in_=plp)
    nc.sync.dma_start(out=arb[0:1, 0:20].rearrange("o (a b) -> (o a) b", a=2),
                      in_=lpT_sb)

    # am/ae/cr + pad: elementwise dot with the w3 row shard
    sc4 = tmp.tile([1, 4], F32, tag="sc4", name="sc4", bufs=1)
    nc.vector.memset(sc4, 0.0)
    for si, nm in enumerate(('am', 'ae', 'cr')):
        w3r = sb(nm + '_w3_s', tmp, tag="brow", bufs=2)
        prod = tmp.tile([1, SH_G], F32, tag="tail", name=nm + "prod")
        nc.vector.tensor_mul(out=prod, in0=h2g[nm], in1=w3r)
        nc.vector.reduce_sum(out=sc4[0:1, si:si + 1], in_=prod, axis=AXX.X)
    nc.sync.dma_start(out=arb[0:1, 20:24], in_=sc4)

    nc.gpsimd.collective_compute(
        "AllReduce", ALU.add, replica_groups=RG,
        ins=[arb[:].opt()], outs=[aro[0:1, :].opt()])

    # ---- tail: logits assembly + masked softmax (replicated) ----
    la = keep.tile([1, 24], F32)
    nc.sync.dma_start(out=la, in_=aro[0:1, :])
    logits = keep.tile([1, 22], F32)
    nc.vector.tensor_copy(out=logits, in_=la[0:1, 0:22])
    mb = sb('maskbias22', keep)
    nc.vector.tensor_add(out=logits, in0=logits, in1=mb)
    mx = keep.tile([1, 1], F32, name="fmx")
    nc.vector.reduce_max(out=mx, in_=logits, axis=AXX.X)
    nc.vector.tensor_scalar_sub(out=logits, in0=logits, scalar1=mx)
    nc.scalar.activation(out=logits, in_=logits, func=AF.Exp)
    sm = keep.tile([1, 1], F32, name="fsm")
    nc.vector.reduce_sum(out=sm, in_=logits, axis=AXX.X)
    nc.vector.reciprocal(out=sm, in_=sm)
    nc.vector.tensor_scalar_mul(out=logits, in0=logits, scalar1=sm)
    nc.sync.dma_start(out=probs_out, in_=logits)
    crb3 = sb('cr_b3', keep)
    val = keep.tile([1, 1], F32, name="val")
    nc.vector.tensor_add(out=val, in0=la[0:1, 22:23], in1=crb3)
    nc.sync.dma_start(out=value_out, in_=val)


def build_program(n_cores=NCORES):
    nc = bacc.Bacc("TRN2", target_bir_lowering=False, debug=False,
                   num_devices=n_cores)
    ins = {}
    for name, shape in INPUT_SPEC.items():
        dt = F32R if name in R32_INPUTS else F32
        ins[name] = nc.dram_tensor(name, list(shape), dt, kind="ExternalInput").ap()
    probs = nc.dram_tensor("probs", [1, 22], F32, kind="ExternalOutput").ap()
    value = nc.dram_tensor("value", [1, 1], F32, kind="ExternalOutput").ap()
    with tile.TileContext(nc) as tc:
        with ExitStack() as ctx:
            emit(ctx, tc, ins, probs, value, n_cores=n_cores)
    nc.compile()
    return nc


_PROG = {}


def _get_program(n_cores=NCORES):
    if n_cores not in _PROG:
        _PROG[n_cores] = build_program(n_cores)
    return _PROG[n_cores]


def kernel(**inputs):
    per_core = host_prep(**inputs)
    nc = _get_program(NCORES)
    res = run_bass_kernel_spmd(nc, per_core, core_ids=list(range(NCORES)))
    out = res.results[0]
    probs = np.asarray(out['probs'], np.float32).reshape(22)
    value = np.asarray(out['value'], np.float32).reshape(1)
    return probs, value


# revision 16
# speedup vs baseline: 1.3686x; 1.1085x over previous
"""Trainium2 Bass kernel for nn_ActorCritic forward (8-core tensor-parallel).

Strategy (memory-bound problem — ~573MB of head weights dominate):
  - Trunk (embedding + one shared transformer block over the 55 concatenated
    hand/draw/disc tokens with block-diagonal attention masking, other-MLP,
    global feature assembly + layernorms) is small and replicated on all
    8 cores.
  - Head MLPs (ac: 4608^2 x2, am/ae/cr: 4096^2 x2) are column-parallel
    across the 8 cores: each core computes a 1/8 slice of
    h1 = relu(W1 x + b1), one AllGather rebuilds h1 everywhere, each core
    computes its h2 slice, then each core contracts its h2 slice with its
    W3 column-shard and a tiny AllReduce (24 floats) sums the partial
    logits/value. The masked softmax tail runs redundantly on every core.
  - Weights ride the PE's MOVING operand (rhs) so weight bytes stream
    HBM->SBUF->PE at DMA rate; weight DMAs are batched 4 K-tiles per
    dma_start (~1MB each) and alternate between the two HWDGE rings
    (sync + scalar engines) to amortize issue cost.
  - The lnac affine is folded into ac W1 on the host; head-layer biases are
    rank-1 matmul accumulates; W3 biases are folded into the tail mask add.
"""

import numpy as np
from contextlib import ExitStack

import concourse.bass as bass
import concourse.bacc as bacc
import concourse.tile as tile
from concourse import mybir
from concourse.bass_utils import run_bass_kernel_spmd
from concourse.masks import make_identity

F32 = mybir.dt.float32
AF = mybir.ActivationFunctionType
ALU = mybir.AluOpType
AXX = mybir.AxisListType
F32R = mybir.dt.float32r


def _r(ap):
    """Tiles feeding the big matmuls are allocated as float32r natively."""
    return ap

NCORES = 8
D = 512
S_HAND, S_DRAW, S_DISC = 10, 30, 15
S = S_HAND + S_DRAW + S_DISC          # 55
NH, HD = 8, 64
EPS = 1e-5
D_AC = 9 * D                           # 4608
D_G = 8 * D                            # 4096
SH_AC = D_AC // NCORES                 # 576
SH_G = D_G // NCORES                   # 512
NEG = -1e30
BN = 7296                              # AG1 bounce: 10*576 + 3*512
OFF_AC, OFF_AM, OFF_AE, OFF_CR = 0, 5760, 6272, 6784
AC_BLK = [(j * 128, min(128, SH_AC - j * 128)) for j in range((SH_AC + 127) // 128)]

# inputs that feed the PE as float32r (full-rate fp32 matmul mode)
R32_INPUTS = {
    'cardsT', 'otherT', 'emb_wT', 'emb_b', 'inproj_wT', 'inproj_b',
    'outproj_wT', 'outproj_b', 'ff1_wT', 'ff1_b', 'ff2_wT', 'ff2_b',
    'other_w1T', 'other_b1', 'other_w2T', 'other_b2',
    'ac_w1T_s', 'ac_b1_s', 'ac_w2T_s', 'ac_b2_s',
    'am_w1T_s', 'am_b1_s', 'am_w2T_s', 'am_b2_s',
    'ae_w1T_s', 'ae_b1_s', 'ae_w2T_s', 'ae_b2_s',
    'cr_w1T_s', 'cr_b1_s', 'cr_w2T_s', 'cr_b2_s',
}

INPUT_SPEC = {
    'cardsT': (128, S), 'sel': (S, 6), 'maskfull': (S, S), 'maskbias22': (1, 22),
    'otherT': (120, 1), 'cr_b3': (1, 1),
    'emb_wT': (128, D), 'emb_b': (1, D),
    'inproj_wT': (D, 3 * D), 'inproj_b': (1, 3 * D),
    'outproj_wT': (D, D), 'outproj_b': (1, D),
    'ln1_g': (D,), 'ln1_b': (D,),
    'ff1_wT': (D, D), 'ff1_b': (1, D),
    'ff2_wT': (D, D), 'ff2_b': (1, D),
    'ln2_g': (D,), 'ln2_b': (D,),
    'other_w1T': (120, 2 * D), 'other_b1': (1, 2 * D),
    'other_w2T': (2 * D, 2 * D), 'other_b2': (1, 2 * D),
    'lng_g': (D_G,), 'lng_b': (D_G,),
    'ac_w1T_s': (D_AC * SH_AC,), 'ac_b1_s': (1, SH_AC),
    'ac_w2T_s': (D_AC * SH_AC,), 'ac_b2_s': (1, SH_AC),
    'ac_w3T_s': (SH_AC, 2),
    'am_w1T_s': (D_G * SH_G,), 'am_b1_s': (1, SH_G),
    'am_w2T_s': (D_G * SH_G,), 'am_b2_s': (1, SH_G),
    'am_w3_s': (1, SH_G),
    'ae_w1T_s': (D_G * SH_G,), 'ae_b1_s': (1, SH_G),
    'ae_w2T_s': (D_G * SH_G,), 'ae_b2_s': (1, SH_G),
    'ae_w3_s': (1, SH_G),
    'cr_w1T_s': (D_G * SH_G,), 'cr_b1_s': (1, SH_G),
    'cr_w2T_s': (D_G * SH_G,), 'cr_b2_s': (1, SH_G),
    'cr_w3_s': (1, SH_G),
}


def _pack_uniform(wT):
    """[K, d] with K = nb*512 -> flat [nb][128][4*d] (contiguous 4-tile DMAs)."""
    K, d = wT.shape
    nb = K // 512
    return np.ascontiguousarray(
        wT.reshape(nb, 4, 128, d).transpose(0, 2, 1, 3)).reshape(-1)


def _pack_ac_blocks(wT):
    """[4608, 576] -> per 576-row block: [128][4*576] batch + [64*576] tail."""
    parts = []
    for c in range(NCORES):
        blk = wT[c * SH_AC:(c + 1) * SH_AC]
        parts.append(np.ascontiguousarray(
            blk[0:512].reshape(4, 128, SH_AC).transpose(1, 0, 2)).reshape(-1))
        parts.append(np.ascontiguousarray(blk[512:SH_AC]).reshape(-1))
    return np.concatenate(parts)


def host_prep(hand, draw_pile, disc_pile, character, monster, energy, params,
              hand_size, valid_action_mask):
    """Build the 8 per-core device input dicts from the full inputs."""
    p = {k: np.asarray(v, np.float32) for k, v in params.items()}
    f32 = np.float32

    cards = np.concatenate([np.asarray(hand, f32), np.asarray(draw_pile, f32),
                            np.asarray(disc_pile, f32)], 0)
    cardsT = np.ascontiguousarray(cards.T)

    maskfull = np.full((S, S), NEG, f32)
    maskfull[:S_HAND, :int(hand_size)] = 0.0
    maskfull[S_HAND:S_HAND + S_DRAW, S_HAND:S_HAND + S_DRAW] = 0.0
    maskfull[S_HAND + S_DRAW:, S_HAND + S_DRAW:] = 0.0

    sel = np.zeros((S, 6), f32)
    sel[:S_HAND, 0] = 1.0 / S_HAND
    sel[:S_HAND, 1] = 1.0
    sel[S_HAND:S_HAND + S_DRAW, 2] = 1.0 / S_DRAW
    sel[S_HAND:S_HAND + S_DRAW, 3] = 1.0
    sel[S_HAND + S_DRAW:, 4] = 1.0 / S_DISC
    sel[S_HAND + S_DRAW:, 5] = 1.0

    # W3 biases are folded into the mask-add before the softmax
    maskbias22 = np.where(np.asarray(valid_action_mask) == 0, NEG, 0.0).astype(f32)
    maskbias22[0:10] += p['ac_b3'][0]
    maskbias22[10:20] += p['ac_b3'][1]
    maskbias22[20] += p['am_b3'][0]
    maskbias22[21] += p['ae_b3'][0]

    # fold the lnac affine into the ac head's first layer:
    #   W1 @ (g*xn + b) + b1 == (W1*g) @ xn + (b1 + W1 @ b)
    ac_w1g = p['ac_w1'] * p['lnac_g'][None, :]
    ac_b1f = p['ac_b1'] + p['ac_w1'] @ p['lnac_b']

    shared = dict(
        cardsT=cardsT, sel=sel, maskfull=maskfull, maskbias22=maskbias22[None, :],
        otherT=np.concatenate([np.asarray(character, f32), np.asarray(monster, f32),
                               np.asarray(energy, f32)])[:, None],
        cr_b3=p['cr_b3'][None, :],
        emb_wT=np.ascontiguousarray(p['emb_card_w'].T), emb_b=p['emb_card_b'][None, :],
        inproj_wT=np.ascontiguousarray(p['in_proj_w'].T), inproj_b=p['in_proj_b'][None, :],
        outproj_wT=np.ascontiguousarray(p['out_proj_w'].T), outproj_b=p['out_proj_b'][None, :],
        ln1_g=p['ln1_g'], ln1_b=p['ln1_b'],
        ff1_wT=np.ascontiguousarray(p['ff1_w'].T), ff1_b=p['ff1_b'][None, :],
        ff2_wT=np.ascontiguousarray(p['ff2_w'].T), ff2_b=p['ff2_b'][None, :],
        ln2_g=p['ln2_g'], ln2_b=p['ln2_b'],
        other_w1T=np.ascontiguousarray(p['other_w1'].T), other_b1=p['other_b1'][None, :],
        other_w2T=np.ascontiguousarray(p['other_w2'].T), other_b2=p['other_b2'][None, :],
        lng_g=p['lng_g'], lng_b=p['lng_b'],
    )
    w1 = {'ac': ac_w1g, 'am': p['am_w1'], 'ae': p['ae_w1'], 'cr': p['cr_w1']}
    b1 = {'ac': ac_b1f, 'am': p['am_b1'], 'ae': p['ae_b1'], 'cr': p['cr_b1']}

    per_core = []
    for c in range(NCORES):
        m = dict(shared)
        for n, sh in (('ac', SH_AC), ('am', SH_G), ('ae', SH_G), ('cr', SH_G)):
            sl = slice(c * sh, (c + 1) * sh)
            m[n + '_w1T_s'] = _pack_uniform(w1[n].T[:, sl])
            m[n + '_b1_s'] = b1[n][None, sl]
            w2s = p[n + '_w2'].T[:, sl]
            m[n + '_w2T_s'] = (_pack_ac_blocks(w2s) if n == 'ac'
                               else _pack_uniform(w2s))
            m[n + '_b2_s'] = p[n + '_b2'][None, sl]
            if n == 'ac':
                m['ac_w3T_s'] = np.ascontiguousarray(p['ac_w3'].T[sl, :])
            else:
                m[n + '_w3_s'] = p[n + '_w3'][:, sl].reshape(1, sh)
        per_core.append(m)
    return per_core


def emit(ctx: ExitStack, tc: tile.TileContext, ins, probs_out, value_out,
         n_cores=NCORES, skip_affine=()):
    nc = tc.nc
    RG = [list(range(n_cores))]

    consts = ctx.enter_context(tc.tile_pool(name="consts", bufs=1))
    keep = ctx.enter_context(tc.tile_pool(name="keep", bufs=1))
    tmp = ctx.enter_context(tc.tile_pool(name="tmp", bufs=2))
    ps = ctx.enter_context(tc.tile_pool(name="ps", bufs=3, space="PSUM"))
    pst = ctx.enter_context(tc.tile_pool(name="pst", bufs=2, space="PSUM"))
    psv = ctx.enter_context(tc.tile_pool(name="psv", bufs=2, space="PSUM"))
    dram = ctx.enter_context(tc.tile_pool(name="dram", bufs=1, space="DRAM"))

    ident = consts.tile([128, 128], F32)
    make_identity(nc, ident)
    ones_f = consts.tile([1, 64], F32)
    nc.vector.memset(ones_f, 1.0)
    ones = consts.tile([1, 64], F32R)
    nc.vector.tensor_copy(out=ones, in_=ones_f)
    eps55 = consts.tile([S, 1], F32)
    nc.vector.memset(eps55, EPS)

    # alternate big weight DMAs across the two HWDGE rings
    _dma_eng = [0]

    def wdma(out, in_):
        eng = nc.sync if _dma_eng[0] % 2 == 0 else nc.scalar
        _dma_eng[0] += 1
        eng.dma_start(out=out, in_=in_)

    def sb(name, pool, tag=None, bufs=None):
        shp = list(INPUT_SPEC[name])
        if len(shp) == 1:
            shp = [1] + shp
        dt = F32R if name in R32_INPUTS else F32
        t = pool.tile(shp, dt, tag=tag or name, name=name + "_sb", bufs=bufs)
        src = ins[name]
        if len(INPUT_SPEC[name]) == 1:
            src = src.rearrange("(o n) -> o n", o=1)
        nc.sync.dma_start(out=t, in_=src)
        return t

    def trans(in_ap, P, Fr):
        pt = pst.tile([Fr, P], F32, tag="ptr", name="ptr")
        nc.tensor.transpose(pt, in_ap, ident[0:P, 0:P])
        return pt

    def ln_normalize(x, P, Dm):
        nsub = Dm // 512
        stats = tmp.tile([P, nsub, 6], F32, tag="lnstats", name="lnstats")
        xs = x.rearrange("p (n d) -> p n d", n=nsub) if nsub > 1 else x
        for i in range(nsub):
            nc.vector.bn_stats(out=stats[:, i, :],
                               in_=(xs[:, i, :] if nsub > 1 else xs))
        mv = tmp.tile([P, 2], F32, tag="lnmv", name="lnmv")
        nc.vector.bn_aggr(out=mv, in_=stats)
        nc.scalar.activation(out=mv[:, 1:2], in_=mv[:, 1:2], func=AF.Sqrt,
                             bias=eps55[0:P], scale=1.0)
        nc.vector.reciprocal(out=mv[:, 1:2], in_=mv[:, 1:2])
        nc.vector.tensor_scalar(out=x, in0=x, scalar1=mv[:, 0:1], scalar2=mv[:, 1:2],
                                op0=ALU.subtract, op1=ALU.mult)

    def ln_affine_chunked(x, P, Dm, g_name, b_name, pool):
        for j in range(Dm // 512):
            gt = pool.tile([P, 512], F32, tag="lnaff", name="lnaff_g", bufs=3)
            bt = pool.tile([P, 512], F32, tag="lnaff", name="lnaff_b", bufs=3)
            for t, nm in ((gt, g_name), (bt, b_name)):
                src = ins[nm][j * 512:(j + 1) * 512]
                ap = bass.AP(tensor=src.tensor, offset=src.offset,
                             ap=[[0, P]] + src.ap)
                nc.gpsimd.dma_start(out=t, in_=ap)
            sl = x[:, j * 512:(j + 1) * 512]
            nc.vector.tensor_mul(out=sl, in0=sl, in1=gt)
            nc.vector.tensor_add(out=sl, in0=sl, in1=bt)

    acT = keep.tile([128, 36 * S_HAND], F32R)       # ac_in^T K-tiles
    xgT = keep.tile([128, 32], F32R)                # x_global^T K-tiles
    xg_d = dram.tile([1, D_G], F32)

    # ================= trunk + W1 (scoped pools) =================
    w1scope = ExitStack()
    wac = w1scope.enter_context(tc.tile_pool(name="wac", bufs=4))
    wg = w1scope.enter_context(tc.tile_pool(name="wg", bufs=3))
    with tc.tile_pool(name="trunk", bufs=1) as trunk, \
         tc.tile_pool(name="ttmp", bufs=2) as ttmp, \
         tc.tile_pool(name="wtrunk", bufs=2) as wtrunk:

        def act55(name):
            return trunk.tile([S, D], F32, tag="act55", name=name, bufs=3)

        cardsT = sb('cardsT', trunk)
        emb_wT = sb('emb_wT', trunk)
        emb_b = sb('emb_b', trunk, tag="brow", bufs=3)
        px = ps.tile([S, D], F32, tag="ps", name="px")
        nc.tensor.matmul(px, lhsT=_r(cardsT), rhs=_r(emb_wT), start=True,
                         stop=False)
        nc.tensor.matmul(px, lhsT=_r(ones[0:1, 0:S]), rhs=_r(emb_b), start=False,
                         stop=True)
        X0 = act55("X0")
        nc.vector.tensor_copy(out=X0, in_=px)

        X0T = trunk.tile([128, 4 * S], F32R)
        for k in range(4):
            pt = trans(X0[:, k * 128:(k + 1) * 128], S, 128)
            nc.vector.tensor_copy(out=X0T[:, k * S:(k + 1) * S], in_=pt)

        inproj_b = sb('inproj_b', trunk, tag='ipb', bufs=1)
        pqkv = [ps.tile([S, D], F32, tag="ps", name=f"pqkv{qi}") for qi in range(3)]
        for k in range(4):
            wt = wtrunk.tile([128, 3 * D], F32R, tag="tw", name="tw")
            wdma(wt, ins['inproj_wT'][k * 128:(k + 1) * 128, :])
            for qi in range(3):
                nc.tensor.matmul(pqkv[qi], lhsT=_r(X0T[:, k * S:(k + 1) * S]),
                                 rhs=_r(wt[:, qi * D:(qi + 1) * D]),
                                 start=(k == 0), stop=False)
        QKV = []
        for qi in range(3):
            nc.tensor.matmul(pqkv[qi], lhsT=_r(ones[0:1, 0:S]),
                             rhs=_r(inproj_b[:, qi * D:(qi + 1) * D]),
                             start=False, stop=True)
            t = trunk.tile([S, D], F32, tag=f"qkv{qi}", name=f"qkv{qi}")
            nc.vector.tensor_copy(out=t, in_=pqkv[qi])
            QKV.append(t)
        Q, K, V = QKV

        # attention: full-Q/K transposes, per-head scores, batched softmax
        qTf = trunk.tile([128, 4 * S], F32)
        kTf = trunk.tile([128, 4 * S], F32)
        for k in range(4):
            nc.vector.tensor_copy(out=qTf[:, k * S:(k + 1) * S],
                                  in_=trans(Q[:, k * 128:(k + 1) * 128], S, 128))
            nc.vector.tensor_copy(out=kTf[:, k * S:(k + 1) * S],
                                  in_=trans(K[:, k * 128:(k + 1) * 128], S, 128))

        maskfull = sb('maskfull', trunk)
        SALL = trunk.tile([S, NH, S], F32)
        for h in range(NH):
            bp = (h % 2) * HD
            blk = slice((h // 2) * S, (h // 2 + 1) * S)
            psc = pst.tile([S, S], F32, tag="ptr", name="psc")
            nc.tensor.matmul(psc, lhsT=qTf[bp:bp + HD, blk],
                             rhs=kTf[bp:bp + HD, blk], start=True, stop=True)
            nc.scalar.activation(out=SALL[:, h, :], in_=psc, func=AF.Copy,
                                 scale=0.125)
        mfap = maskfull[:]
        nc.vector.tensor_add(
            out=SALL, in0=SALL,
            in1=bass.AP(tensor=mfap.tensor, offset=mfap.offset,
                        ap=[mfap.ap[0], [0, NH], mfap.ap[1]]))
        mx8 = ttmp.tile([S, NH], F32, tag="mx8", name="mx8")
        nc.vector.reduce_max(out=mx8, in_=SALL, axis=AXX.X)
        mxap = mx8[:]
        nc.vector.tensor_tensor(
            out=SALL, in0=SALL,
            in1=bass.AP(tensor=mxap.tensor, offset=mxap.offset,
                        ap=[mxap.ap[0], mxap.ap[1], [0, S]]),
            op=ALU.subtract)
        nc.scalar.activation(out=SALL, in_=SALL, func=AF.Exp)
        sm8 = ttmp.tile([S, NH], F32, tag="sm8", name="sm8")
        nc.vector.reduce_sum(out=sm8, in_=SALL, axis=AXX.X)
        nc.vector.reciprocal(out=sm8, in_=sm8)
        smap = sm8[:]
        nc.vector.tensor_tensor(
            out=SALL, in0=SALL,
            in1=bass.AP(tensor=smap.tensor, offset=smap.offset,
                        ap=[smap.ap[0], smap.ap[1], [0, S]]),
            op=ALU.mult)
        AOp = ps.tile([S, D], F32, tag="ao", name="AOp", bufs=1)
        for h in range(NH):
            hs = slice(h * HD, (h + 1) * HD)
            aT = ttmp.tile([S, S], F32, tag="aT", name="aT")
            nc.vector.tensor_copy(out=aT, in_=trans(SALL[:, h, :], S, S))
            nc.tensor.matmul(AOp[:, hs], lhsT=aT, rhs=V[:, hs], start=True,
                             stop=True)
        AO = act55("AO")
        nc.vector.tensor_copy(out=AO, in_=AOp)

        def ln_bcast(x, P, g_name, b_name):
            ln_normalize(x, P, D)
            if g_name not in skip_affine:
                ln_affine_chunked(x, P, D, g_name, b_name, ttmp)

        def mlp512(x_sb, wT_name, b_name, out_name, extra_add=None):
            xT = trunk.tile([128, 4 * S], F32R, tag="xT", name=wT_name + "_xT",
                            bufs=2)
            for k in range(4):
                pt = trans(x_sb[:, k * 128:(k + 1) * 128], S, 128)
                nc.vector.tensor_copy(out=xT[:, k * S:(k + 1) * S], in_=pt)
            b_sb = sb(b_name, trunk, tag="brow", bufs=3)
            po = ps.tile([S, D], F32, tag="ps", name=wT_name + "_po")
            wt = wtrunk.tile([128, 4, D], F32R, tag="tw2", name=wT_name + "_w")
            wdma(wt, ins[wT_name][:].rearrange("(a p) d -> p a d", p=128))
            for k in range(4):
                nc.tensor.matmul(po, lhsT=_r(xT[:, k * S:(k + 1) * S]),
                                 rhs=_r(wt[:, k, :]), start=(k == 0), stop=False)
            nc.tensor.matmul(po, lhsT=_r(ones[0:1, 0:S]), rhs=_r(b_sb),
                             start=False, stop=True)
            o = act55(out_name)
            if extra_add is not None:
                nc.vector.tensor_add(out=o, in0=po, in1=extra_add)
            else:
                nc.vector.tensor_copy(out=o, in_=po)
            return o

        X1 = mlp512(AO, 'outproj_wT', 'outproj_b', "X1", extra_add=X0)
        ln_bcast(X1, S, 'ln1_g', 'ln1_b')
        Rf = mlp512(X1, 'ff1_wT', 'ff1_b', "Rf")
        nc.scalar.activation(out=Rf, in_=Rf, func=AF.Relu)
        X2 = mlp512(Rf, 'ff2_wT', 'ff2_b', "X2", extra_add=X1)
        ln_bcast(X2, S, 'ln2_g', 'ln2_b')

        sel = sb('sel', trunk)
        pagg = psv.tile([6, D], F32, tag="psv", name="pagg")
        nc.tensor.matmul(pagg, lhsT=sel, rhs=X2, start=True, stop=True)
        agg_d = dram.tile([1, 6 * D], F32)
        agg_sb = ttmp.tile([6, D], F32, tag="aggsb", name="aggsb")
        nc.vector.tensor_copy(out=agg_sb, in_=pagg)
        nc.sync.dma_start(out=agg_d[0:1, :].rearrange("o (r d) -> (o r) d", r=6),
                          in_=agg_sb)

        otherT = sb('otherT', trunk)
        ob1 = sb('other_b1', trunk)
        ow1 = wtrunk.tile([120, 2 * D], F32R, tag="ow1", name="ow1", bufs=1)
        wdma(ow1, ins['other_w1T'])
        O1 = trunk.tile([1, 2 * D], F32)
        for half in range(2):
            pv = psv.tile([1, D], F32, tag="psv", name="po1")
            nc.tensor.matmul(pv, lhsT=_r(otherT),
                             rhs=_r(ow1[:, half * D:(half + 1) * D]),
                             start=True, stop=False)
            nc.tensor.matmul(pv, lhsT=_r(ones[0:1, 0:1]),
                             rhs=_r(ob1[:, half * D:(half + 1) * D]),
                             start=False, stop=True)
            nc.scalar.activation(out=O1[:, half * D:(half + 1) * D], in_=pv,
                                 func=AF.Relu)
        o1_d = dram.tile([1, 2 * D], F32)
        nc.sync.dma_start(out=o1_d, in_=O1)
        o1n = ttmp.tile([8, 128], F32, tag="o1n", name="o1n")
        nc.sync.dma_start(out=o1n,
                          in_=o1_d[0:1, :].rearrange("o (r p) -> (o r) p", p=128))
        O1T = trunk.tile([128, 8], F32R)
        nc.vector.tensor_copy(out=O1T, in_=trans(o1n, 8, 128))
        ob2 = sb('other_b2', trunk)
        O2 = trunk.tile([1, 2 * D], F32)
        for half in range(2):
            pv = psv.tile([1, D], F32, tag="psv", name="po2")
            for kb in range(2):
                wt = wtrunk.tile([128, 4, D], F32R, tag="tw2", name="ow2")
                wdma(wt, ins['other_w2T'][kb * 512:(kb + 1) * 512,
                                          half * D:(half + 1) * D].rearrange(
                    "(a p) d -> p a d", p=128))
                for a in range(4):
                    nc.tensor.matmul(pv,
                                     lhsT=_r(O1T[:, kb * 4 + a:kb * 4 + a + 1]),
                                     rhs=_r(wt[:, a, :]),
                                     start=(kb == 0 and a == 0), stop=False)
            nc.tensor.matmul(pv, lhsT=_r(ones[0:1, 0:1]),
                             rhs=_r(ob2[:, half * D:(half + 1) * D]),
                             start=False, stop=True)
            nc.scalar.activation(out=O2[:, half * D:(half + 1) * D], in_=pv,
                                 func=AF.Relu)

        XG = trunk.tile([1, D_G], F32, tag="bigx", name="XG", bufs=1)
        nc.sync.dma_start(out=XG[0:1, 0:6 * D], in_=agg_d)
        nc.vector.tensor_copy(out=XG[0:1, 6 * D:8 * D], in_=O2)
        ln_normalize(XG, 1, D_G)
        if 'lng_g' not in skip_affine:
            ln_affine_chunked(XG, 1, D_G, 'lng_g', 'lng_b', ttmp)
        nc.sync.dma_start(out=xg_d, in_=XG)

        ACIN = trunk.tile([S_HAND, D_AC], F32, tag="bigx", name="ACIN", bufs=1)
        xgap = xg_d[:]
        nc.gpsimd.dma_start(out=ACIN[:, 0:D_G],
                            in_=bass.AP(tensor=xgap.tensor, offset=xgap.offset,
                                        ap=[[0, S_HAND]] + xgap.ap[-1:]))
        nc.vector.tensor_copy(out=ACIN[:, D_G:D_AC], in_=X2[0:S_HAND, :])
        ln_normalize(ACIN, S_HAND, D_AC)

        xgn = ttmp.tile([32, 128], F32, tag="xgn", name="xgn")
        nc.sync.dma_start(out=xgn,
                          in_=xg_d[0:1, :].rearrange("o (r p) -> (o r) p", p=128))
        nc.vector.tensor_copy(out=xgT, in_=trans(xgn, 32, 128))
        for k in range(36):
            pt = trans(ACIN[:, k * 128:(k + 1) * 128], S_HAND, 128)
            nc.vector.tensor_copy(out=acT[:, k * S_HAND:(k + 1) * S_HAND], in_=pt)
    # ================= end trunk scope =================

    bounce1 = dram.tile([1, BN], F32)
    gout1 = dram.tile([n_cores, BN], F32)

    def head_ac_layer(lhsT_tiles, w_name, b_name, bounce, pool, blocks):
        """ac head layer. blocks: list of (row0, [(off,sz)...]) DMA batches;
        lhsT_tiles(i, sz) gives the K-tile lhsT AP in running order."""
        b_sb = sb(b_name, tmp, tag="brow", bufs=2)
        HA = SH_AC // 2
        pa = ps.tile([S_HAND, HA], F32, tag="ps", name=w_name + "_pa")
        pb = psv.tile([S_HAND, HA], F32, tag="psv", name=w_name + "_pb")
        i = 0
        first = True
        foff = 0
        for row0, subs in blocks:
            nfull = sum(1 for (_, sz) in subs if sz == 128)
            wt = pool.tile([128, len(subs), SH_AC], F32R, tag="w", name="wt")
            if nfull:
                nel = 128 * nfull * SH_AC
                wdma(wt[:, 0:nfull, :],
                     ins[w_name][foff:foff + nel].rearrange(
                         "(p a d) -> p a d", p=128, a=nfull))
                foff += nel
            if nfull < len(subs):
                off, sz = subs[nfull]
                nel = sz * SH_AC
                wdma(wt[0:sz, nfull, :],
                     ins[w_name][foff:foff + nel].rearrange(
                         "(p d) -> p d", p=sz))
                foff += nel
            for a, (off, sz) in enumerate(subs):
                lt = _r(lhsT_tiles(i, sz))
                nc.tensor.matmul(pa, lhsT=lt, rhs=_r(wt[0:sz, a, 0:HA]),
                                 start=first, stop=False)
                nc.tensor.matmul(pb, lhsT=lt, rhs=_r(wt[0:sz, a, HA:SH_AC]),
                                 start=first, stop=False)
                first = False
                i += 1
        nc.tensor.matmul(pa, lhsT=_r(ones[0:1, 0:S_HAND]), rhs=_r(b_sb[:, 0:HA]),
                         start=False, stop=True)
        nc.tensor.matmul(pb, lhsT=_r(ones[0:1, 0:S_HAND]),
                         rhs=_r(b_sb[:, HA:SH_AC]), start=False, stop=True)
        h = tmp.tile([S_HAND, SH_AC], F32, tag="hy", name="h_ac")
        nc.scalar.activation(out=h[:, 0:HA], in_=pa, func=AF.Relu)
        nc.scalar.activation(out=h[:, HA:SH_AC], in_=pb, func=AF.Relu)
        if bounce is not None:
            nc.sync.dma_start(
                out=bounce[0:1, OFF_AC:OFF_AC + S_HAND * SH_AC].rearrange(
                    "o (t d) -> (o t) d", t=S_HAND),
                in_=h)
        return h

    def head_g_layer(lhsT_tiles, w_name, b_name, bounce, off, pool):
        b_sb = sb(b_name, tmp, tag="brow", bufs=2)
        pv = psv.tile([1, SH_G], F32, tag="psv", name=w_name + "_pv")
        for kb in range(8):
            wt = pool.tile([128, 4, SH_G], F32R, tag="w", name="wt")
            nel = 128 * 4 * SH_G
            wdma(wt, ins[w_name][kb * nel:(kb + 1) * nel].rearrange(
                "(p a d) -> p a d", p=128, a=4))
            for a in range(4):
                nc.tensor.matmul(pv, lhsT=_r(lhsT_tiles(kb * 4 + a)),
                                 rhs=_r(wt[:, a, :]),
                                 start=(kb == 0 and a == 0), stop=False)
        nc.tensor.matmul(pv, lhsT=_r(ones[0:1, 0:1]), rhs=_r(b_sb), start=False,
                         stop=True)
        h = tmp.tile([1, SH_G], F32, tag="h_g", name="h_g")
        nc.scalar.activation(out=h, in_=pv, func=AF.Relu)
        if bounce is not None:
            nc.sync.dma_start(out=bounce[0:1, off:off + SH_G], in_=h)
        return h

    # W1: uniform 36 K-tiles in 9 batches of 4
    w1_blocks = [(b * 512, [(a * 128, 128) for a in range(4)]) for b in range(9)]
    head_ac_layer(lambda i, sz: acT[:, i * S_HAND:(i + 1) * S_HAND], 'ac_w1T_s',
                  'ac_b1_s', bounce1, wac, w1_blocks)
    head_g_layer(lambda k: xgT[:, k:k + 1], 'am_w1T_s', 'am_b1_s', bounce1,
                 OFF_AM, wg)
    head_g_layer(lambda k: xgT[:, k:k + 1], 'ae_w1T_s', 'ae_b1_s', bounce1,
                 OFF_AE, wg)
    head_g_layer(lambda k: xgT[:, k:k + 1], 'cr_w1T_s', 'cr_b1_s', bounce1,
                 OFF_CR, wg)

    nc.gpsimd.collective_compute(
        "AllGather", ALU.bypass, replica_groups=RG,
        ins=[bounce1[:].opt()], outs=[gout1[:].opt()])

    # load + transpose gathered h1 into lhsT K-tiles
    h1acK = keep.tile([128, n_cores * len(AC_BLK) * S_HAND], F32R)
    for c in range(n_cores):
        yc = tmp.tile([S_HAND, SH_AC], F32, tag="hy", name="yc")
        nc.sync.dma_start(
            out=yc,
            in_=gout1[c:c + 1, OFF_AC:OFF_AC + S_HAND * SH_AC].rearrange(
                "o (t d) -> (o t) d", t=S_HAND))
        for j, (off, sz) in enumerate(AC_BLK):
            i = c * len(AC_BLK) + j
            pt = trans(yc[:, off:off + sz], S_HAND, sz)
            nc.vector.tensor_copy(out=h1acK[0:sz, i * S_HAND:(i + 1) * S_HAND],
                                  in_=pt)
    h1gK = {}
    for nm, off in (('am', OFF_AM), ('ae', OFF_AE), ('cr', OFF_CR)):
        gn = tmp.tile([32, 128], F32, tag="gn", name=nm + "gn")
        for c in range(n_cores):
            nc.sync.dma_start(
                out=gn[c * 4:(c + 1) * 4, :],
                in_=gout1[c:c + 1, off:off + SH_G].rearrange(
                    "o (r p) -> (o r) p", p=128))
        t = keep.tile([128, 32], F32R, tag="g1" + nm, name="g1" + nm)
        nc.vector.tensor_copy(out=t, in_=trans(gn, 32, 128))
        h1gK[nm] = t

    w1scope.close()  # release W1 weight pools; W2 pools reuse the space
    wac2 = ctx.enter_context(tc.tile_pool(name="wac2", bufs=10))
    wg2 = ctx.enter_context(tc.tile_pool(name="wg2", bufs=8))

    # W2 (K-tiles follow the 8 gathered blocks: 4x128+64 per block)
    w2_blocks = [(c * SH_AC, AC_BLK) for c in range(n_cores)]
    h2ac = head_ac_layer(
        lambda i, sz: h1acK[0:sz, i * S_HAND:(i + 1) * S_HAND], 'ac_w2T_s',
        'ac_b2_s', None, wac2, w2_blocks)
    h2g = {}
    for nm in ('am', 'ae', 'cr'):
        h2g[nm] = head_g_layer(lambda k, _n=nm: h1gK[_n][:, k:k + 1],
                               nm + '_w2T_s', nm + '_b2_s', None, 0, wg2)

    # ---- W3 partials on local h2 slices + tiny AllReduce ----
    arb = dram.tile([1, 24], F32)
    aro = dram.tile([1, 24], F32)  # AR output

    # ac: lpT_partial [2,10] = w3_sT.T @ h2ac^T over the local 576 dims
    w3s = keep.tile([128, 5, 2], F32)
    nc.sync.dma_start(out=w3s[:, 0:4, :],
                      in_=ins['ac_w3T_s'][0:512, :].rearrange(
                          "(j p) o -> p j o", p=128))
    nc.sync.dma_start(out=w3s[0:64, 4, :], in_=ins['ac_w3T_s'][512:576, :])
    plp = psv.tile([2, S_HAND], F32, tag="psv", name="plp")
    for j, (off, sz) in enumerate(AC_BLK):
        pt = trans(h2ac[:, off:off + sz], S_HAND, sz)
        h2T = tmp.tile([128, S_HAND], F32, tag="h2T", name="h2T")
        nc.vector.tensor_copy(out=h2T[0:sz, :], in_=pt)
        nc.tensor.matmul(plp, lhsT=w3s[0:sz, j, :], rhs=h2T[0:sz, :],
                         start=(j == 0), stop=(j == len(AC_BLK) - 1))
    lpT_sb = tmp.tile([2, S_HAND], F32, tag="tail", name="lpT")
    nc.vector.tensor_copy(out=lpT_sb, in_=plp)
    nc.sync.dma_start(out=arb[0:1, 0:20].rearrange("o (a b) -> (o a) b", a=2),
                      in_=lpT_sb)

    # am/ae/cr + pad: elementwise dot with the w3 row shard
    sc4 = tmp.tile([1, 4], F32, tag="sc4", name="sc4", bufs=1)
    nc.vector.memset(sc4, 0.0)
    for si, nm in enumerate(('am', 'ae', 'cr')):
        w3r = sb(nm + '_w3_s', tmp, tag="brow", bufs=2)
        prod = tmp.tile([1, SH_G], F32, tag="tail", name=nm + "prod")
        nc.vector.tensor_mul(out=prod, in0=h2g[nm], in1=w3r)
        nc.vector.reduce_sum(out=sc4[0:1, si:si + 1], in_=prod, axis=AXX.X)
    nc.sync.dma_start(out=arb[0:1, 20:24], in_=sc4)

    nc.gpsimd.collective_compute(
        "AllReduce", ALU.add, replica_groups=RG,
        ins=[arb[:].opt()], outs=[aro[0:1, :].opt()])

    # ---- tail: logits assembly + masked softmax (replicated) ----
    la = keep.tile([1, 24], F32)
    nc.sync.dma_start(out=la, in_=aro[0:1, :])
    logits = keep.tile([1, 22], F32)
    nc.vector.tensor_copy(out=logits, in_=la[0:1, 0:22])
    mb = sb('maskbias22', keep)
    nc.vector.tensor_add(out=logits, in0=logits, in1=mb)
    mx = keep.tile([1, 1], F32, name="fmx")
    nc.vector.reduce_max(out=mx, in_=logits, axis=AXX.X)
    nc.vector.tensor_scalar_sub(out=logits, in0=logits, scalar1=mx)
    nc.scalar.activation(out=logits, in_=logits, func=AF.Exp)
    sm = keep.tile([1, 1], F32, name="fsm")
    nc.vector.reduce_sum(out=sm, in_=logits, axis=AXX.X)
    nc.vector.reciprocal(out=sm, in_=sm)
    nc.vector.tensor_scalar_mul(out=logits, in0=logits, scalar1=sm)
    nc.sync.dma_start(out=probs_out, in_=logits)
    crb3 = sb('cr_b3', keep)
    val = keep.tile([1, 1], F32, name="val")
    nc.vector.tensor_add(out=val, in0=la[0:1, 22:23], in1=crb3)
    nc.sync.dma_start(out=value_out, in_=val)


def build_program(n_cores=NCORES, skip_affine=()):
    nc = bacc.Bacc("TRN2", target_bir_lowering=False, debug=False,
                   num_devices=n_cores)
    ins = {}
    for name, shape in INPUT_SPEC.items():
        dt = F32R if name in R32_INPUTS else F32
        ins[name] = nc.dram_tensor(name, list(shape), dt, kind="ExternalInput").ap()
    probs = nc.dram_tensor("probs", [1, 22], F32, kind="ExternalOutput").ap()
    value = nc.dram_tensor("value", [1, 1], F32, kind="ExternalOutput").ap()
    with tile.TileContext(nc) as tc:
        with ExitStack() as ctx:
            emit(ctx, tc, ins, probs, value, n_cores=n_cores,
                 skip_affine=skip_affine)
    nc.compile()
    return nc


_PROG = {}


def _get_program(n_cores=NCORES, skip_affine=()):
    key = (n_cores, tuple(sorted(skip_affine)))
    if key not in _PROG:
        _PROG[key] = build_program(n_cores, skip_affine)
    return _PROG[key]


def affine_skip_flags(params):
    """LN affines that are exactly identity can be compiled out."""
    out = []
    for g, b in (('ln1_g', 'ln1_b'), ('ln2_g', 'ln2_b'), ('lng_g', 'lng_b')):
        if (np.all(np.asarray(params[g]) == 1.0)
                and np.all(np.asarray(params[b]) == 0.0)):
            out.append(g)
    return tuple(out)


def kernel(**inputs):
    per_core = host_prep(**inputs)
    nc = _get_program(NCORES, affine_skip_flags(inputs['params']))
    res = run_bass_kernel_spmd(nc, per_core, core_ids=list(range(NCORES)))
    out = res.results[0]
    probs = np.asarray(out['probs'], np.float32).reshape(22)
    value = np.asarray(out['value'], np.float32).reshape(1)
    return probs, value
